# revision 39
# baseline (speedup 1.0000x reference)
"""Trainium2 Bass kernel for nn_Basic_Block_v1 (spatial/spectral Mamba2 block).

Sharding: data-parallel over batch (16 samples) across 8 NeuronCores,
2 samples per core; all parameters replicated. The SSD scans are computed
in closed quadratic form (masked decay matrix x dt-scaled inputs) so all
heavy math runs on the TensorEngine.
"""
import sys
sys.path.insert(0, '/opt/trn_rl_repo')
import json
import os

import numpy as np

import concourse.bass as bass
import concourse.mybir as mybir
from concourse import tile
from concourse.bass_utils import run_bass_kernel_spmd

F32 = mybir.dt.float32
F16 = mybir.dt.float16
I32 = mybir.dt.int32
AF = mybir.ActivationFunctionType
ALU = mybir.AluOpType
AX = mybir.AxisListType

NCORES = 8
BPC = 2          # batch per core
L = 256          # spatial tokens
C = 128          # channels
H1 = 4           # spa heads
DI1 = 256        # spa d_inner
H2 = 8           # spe heads
DI2 = 512        # spe d_inner
L2 = 128         # spe tokens (channels)
DM2 = 256        # spe d_model (seq positions)
NST = 64         # d_state
EPS = 1e-5

# ---------------------------------------------------------------------------
# walrus in this container supports only ONE sync-wait per instruction;
# split extra waits emitted by the Tile scheduler onto preceding NoOps.
_WAIT_LIMIT = 1
_orig_to_json = bass.Bass.to_json_bytes


def _fix_block(b, ctr):
    insts = b.get('instructions')
    if insts:
        out = []
        for ins in insts:
            si = ins.get('sync_info')
            waits = (si or {}).get('on_wait') or []
            if len(waits) > _WAIT_LIMIT:
                while len(waits) > _WAIT_LIMIT:
                    chunk, waits = waits[:_WAIT_LIMIT], waits[_WAIT_LIMIT:]
                    ctr[0] += 1
                    out.append({
                        "debug": ins.get("debug"),
                        "engine": ins["engine"],
                        "ins": [],
                        "name": f"I-wsplit{ctr[0]}",
                        "opcode": "NoOp",
                        "outs": [],
                        "text_hint": "wsplit",
                        "sync_info": {"on_update": [], "on_wait": chunk},
                    })
                si['on_wait'] = waits
            out.append(ins)
        b['instructions'] = out
    for sb in b.get('blocks') or []:
        _fix_block(sb, ctr)


def _patched_to_json(self, *a, **k):
    raw = _orig_to_json(self, *a, **k)
    d = json.loads(raw)
    ctr = [0]
    for f in d.get('functions', []):
        for b in f.get('blocks', []):
            _fix_block(b, ctr)
    if ctr[0] == 0:
        return raw
    return json.dumps(d).encode()


bass.Bass.to_json_bytes = _patched_to_json


# ---------------------------------------------------------------------------
def _sincos_2d(dim, Hg):
    def e1(d, pos):
        omega = 1.0 / (10000.0 ** (np.arange(d // 2, dtype=np.float64) / (d / 2.0)))
        out = pos[:, None] * omega[None, :]
        return np.concatenate([np.sin(out), np.cos(out)], axis=-1)
    gh, gw = np.meshgrid(np.arange(Hg), np.arange(Hg), indexing='ij')
    emb = np.concatenate([e1(dim // 2, gh.reshape(-1)), e1(dim // 2, gw.reshape(-1))], axis=-1)
    return emb.astype(np.float32)


def host_constants():
    d = {}
    d['pe_fm'] = np.ascontiguousarray(_sincos_2d(C, 16).T)              # [128, 256]
    d['ident'] = np.eye(128, dtype=np.float32)
    d['identh'] = np.eye(128, dtype=np.float16)
    iota = np.arange(L, dtype=np.float32)
    d['iotaC'] = np.stack([iota[:128], iota[128:]], axis=1).copy()      # [128, 2]
    # maskT[st][sp][t] = 1 if (st*128+sp) <= t   (spa, L=256)
    sidx = np.arange(L)[:, None]
    tidx = np.arange(L)[None, :]
    m = (sidx <= tidx).astype(np.float32)                               # [s, t]
    d['maskT_spa'] = np.stack([m[:128], m[128:]], axis=1).copy()        # [128, 2, 256]
    s2 = np.arange(L2)[:, None]
    t2 = np.arange(L2)[None, :]
    d['maskT_spe'] = (s2 <= t2).astype(np.float32)                      # [128, 128]
    # head one-hots for dt broadcast: E[k, j, m] = 1 iff k == 2j + m//64
    E1 = np.zeros((H1, 2, 128), np.float32)
    for j in range(2):
        for m in range(128):
            E1[2 * j + m // 64, j, m] = 1.0
    d['E_spaJ'] = E1.astype(np.float16)
    E2 = np.zeros((H2, 4, 128), np.float32)
    for j in range(4):
        for m in range(128):
            E2[2 * j + m // 64, j, m] = 1.0
    d['E_speJ'] = E2.astype(np.float16)
    EA = np.zeros((8, 128), np.float32)
    for h in range(8):
        EA[h, h * 16:(h + 1) * 16] = 1.0
    d['E_attn'] = EA.astype(np.float16)                                 # [8, 128]

    d['Emask_q'] = EA.T.copy()                                          # [128, 8]
    d['onesrow'] = np.ones(512, np.float16)
    return d


COL_ORDER = (
    ["spa_dtb0", "spa_alog0", "spa_cb0_0", "spa_cb0_1", "spa_cbBC0",
     "spa_dpc0_0", "spa_dpc0_1", "spa_rwc0_0", "spa_rwc0_1",
     "spa_dtb1", "spa_alog1", "spa_cb1_0", "spa_cb1_1", "spa_cbBC1",
     "spa_dpc1_0", "spa_dpc1_1", "spa_rwc1_0", "spa_rwc1_1"]
    + ["spe_dtb0", "spe_alog0", "spe_cb0_0", "spe_cb0_1", "spe_cb0_2", "spe_cb0_3",
       "spe_cbBC0",
       "spe_dpc0_0", "spe_dpc0_1", "spe_dpc0_2", "spe_dpc0_3",
       "spe_rwc0_0", "spe_rwc0_1", "spe_rwc0_2", "spe_rwc0_3",
       "spe_dtb1", "spe_alog1", "spe_cb1_0", "spe_cb1_1", "spe_cb1_2", "spe_cb1_3",
       "spe_cbBC1",
       "spe_dpc1_0", "spe_dpc1_1", "spe_dpc1_2", "spe_dpc1_3",
       "spe_rwc1_0", "spe_rwc1_1", "spe_rwc1_2", "spe_rwc1_3"]
    + ["lnw_spa0", "lnw_spa1", "lnw_norm",
       "cprj_b", "aq_b", "ak_b", "av_b", "ao_b",
       "sq_b0", "sq_b1", "sk_b0", "sk_b1"]
)
CIDX = {k: ix for ix, k in enumerate(COL_ORDER)}


F16_WEIGHTS = (
    'spa_in_wT', 'spa_out_pk', 'spe_in_pk', 'spe_out_pk', 'cprj_pk',
    'aqT', 'akT', 'avT', 'aoT', 'sqT', 'skT', 'svT', 'soT', 'dsw_pk', 'lnwb')


def prep_weights(inp):
    """Host-side layout prep of the replicated parameters (tile layouts,
    single DMA per tensor)."""
    w = {}
    w['spa_in_wT'] = np.ascontiguousarray(np.transpose(inp['spa_in_w'], (0, 2, 1)))
    cv = np.zeros((128, 2, 3, 4), np.float32)
    for i in range(2):
        cv[:, i, 0] = inp['spa_conv_w'][i, 0:128]
        cv[:, i, 1] = inp['spa_conv_w'][i, 128:256]
        cv[0:64, i, 2] = inp['spa_conv_w'][i, 256:320]
        cv[64:128, i, 2] = inp['spa_conv_w'][i, 320:384]
    w['spa_conv_pk'] = cv
    sow = np.transpose(inp['spa_out_w'], (0, 2, 1)).reshape(2, 2, 128, 128)
    w['spa_out_pk'] = np.ascontiguousarray(sow.transpose(2, 0, 1, 3))
    w['spe_ln_wB'] = np.ascontiguousarray(np.broadcast_to(
        inp['spe_ln_w'][:, None, :], (2, 128, 256)).transpose(1, 0, 2))
    w['spe_ln_bB'] = np.ascontiguousarray(np.broadcast_to(
        inp['spe_ln_b'][:, None, :], (2, 128, 256)).transpose(1, 0, 2))
    siw = np.transpose(inp['spe_in_w'], (0, 2, 1)).reshape(2, 2, 128, 1160)
    w['spe_in_pk'] = np.ascontiguousarray(siw.transpose(0, 2, 1, 3))
    cv2 = np.zeros((128, 2, 5, 4), np.float32)
    for i in range(2):
        for j in range(4):
            cv2[:, i, j] = inp['spe_conv_w'][i, j * 128:(j + 1) * 128]
        cv2[0:64, i, 4] = inp['spe_conv_w'][i, 512:576]
        cv2[64:128, i, 4] = inp['spe_conv_w'][i, 576:640]
    w['spe_conv_pk'] = cv2
    sew = np.transpose(inp['spe_out_w'], (0, 2, 1)).reshape(2, 4, 128, 256)
    w['spe_out_pk'] = np.ascontiguousarray(sew.transpose(0, 2, 1, 3))
    w['cprj_pk'] = np.ascontiguousarray(
        np.transpose(inp['cprj_w'], (2, 1, 0)).transpose(1, 0, 2))
    for nm in ('aq', 'ak', 'av', 'ao'):
        w[nm + 'T'] = np.ascontiguousarray(inp[nm + '_w'].T)
    for nm in ('sq', 'sk', 'sv', 'so'):
        wt_ = inp[nm + '_w'].T.reshape(2, 128, 256)
        w[nm + 'T'] = np.ascontiguousarray(wt_.transpose(1, 0, 2))
    w['svbB'] = np.ascontiguousarray(np.broadcast_to(inp['sv_b'][None, :], (128, 256)))
    w['sobB'] = np.ascontiguousarray(np.broadcast_to(inp['so_b'][None, :], (128, 256)))
    w['dsw_pk'] = np.ascontiguousarray(
        inp['ds_conv_w'].reshape(9, 128, 128).transpose(1, 0, 2))
    w['ds_ln_wB'] = np.ascontiguousarray(np.broadcast_to(inp['ds_ln_w'][None, :], (64, 128)))
    w['ds_ln_bB'] = np.ascontiguousarray(np.broadcast_to(inp['ds_ln_b'][None, :], (64, 128)))
    lnwb = np.zeros((2, 3, 128), np.float32)
    lnwb[0, 0], lnwb[1, 0] = inp['spa_ln_w'][0], inp['spa_ln_b'][0]
    lnwb[0, 1], lnwb[1, 1] = inp['spa_ln_w'][1], inp['spa_ln_b'][1]
    lnwb[0, 2], lnwb[1, 2] = inp['norm_w'], inp['norm_b']
    w['lnwb'] = lnwb
    cols = {}
    for i in range(2):
        cols[f"spa_dtb{i}"] = inp['spa_dt_bias'][i]
        cols[f"spa_alog{i}"] = np.exp(inp['spa_A_log'][i])
        cols[f"spa_cb{i}_0"] = inp['spa_conv_b'][i, 0:128]
        cols[f"spa_cb{i}_1"] = inp['spa_conv_b'][i, 128:256]
        cols[f"spa_cbBC{i}"] = inp['spa_conv_b'][i, 256:384]
        for j in range(2):
            cols[f"spa_dpc{i}_{j}"] = np.repeat(inp['spa_D'][i], 64)[j * 128:(j + 1) * 128]
            cols[f"spa_rwc{i}_{j}"] = inp['spa_rms_w'][i, j * 128:(j + 1) * 128]
        cols[f"spe_dtb{i}"] = inp['spe_dt_bias'][i]
        cols[f"spe_alog{i}"] = np.exp(inp['spe_A_log'][i])
        for j in range(4):
            cols[f"spe_cb{i}_{j}"] = inp['spe_conv_b'][i, j * 128:(j + 1) * 128]
            cols[f"spe_dpc{i}_{j}"] = np.repeat(inp['spe_D'][i], 64)[j * 128:(j + 1) * 128]
            cols[f"spe_rwc{i}_{j}"] = inp['spe_rms_w'][i, j * 128:(j + 1) * 128]
        cols[f"spe_cbBC{i}"] = inp['spe_conv_b'][i, 512:640]
    cols["lnw_spa0"] = inp['spa_ln_w'][0]
    cols["lnw_spa1"] = inp['spa_ln_w'][1]
    cols["lnw_norm"] = inp['norm_w']
    cols["cprj_b"] = inp['cprj_b']
    for nm in ('aq', 'ak', 'av', 'ao'):
        cols[nm + "_b"] = inp[nm + '_b']
    cols["sq_b0"] = inp['sq_b'][0:128]
    cols["sq_b1"] = inp['sq_b'][128:256]
    cols["sk_b0"] = inp['sk_b'][0:128]
    cols["sk_b1"] = inp['sk_b'][128:256]
    pk = np.zeros((128, len(COL_ORDER)), np.float32)
    for k, v in cols.items():
        v = np.asarray(v, np.float32)
        pk[0:v.shape[0], CIDX[k]] = v
    w['colpak'] = pk
    for k in F16_WEIGHTS:
        w[k] = w[k].astype(np.float16)
    return w



# ---------------------------------------------------------------------------
def build_program(taps=()):
    """Builds the per-core SPMD Bass program. `taps` is a set of intermediate
    names to also write to DRAM outputs (debug only)."""
    nc = bass.Bass()

    def din(name, shape, dt=F32):
        return nc.dram_tensor(name, shape, dt, kind="ExternalInput")

    x2 = din("x2", [BPC, C, L])
    idx = din("idx", [BPC, L], I32)
    inv = din("inv", [BPC, L], I32)

    cst = host_constants()
    cst_t = {k: din(k, list(v.shape), F16 if v.dtype == np.float16 else F32)
             for k, v in cst.items()}

    wnames = {
        'spa_in_wT': [2, 128, 644], 'spa_conv_pk': [128, 2, 3, 4],
        'spa_out_pk': [128, 2, 2, 128],
        'spe_ln_wB': [128, 2, 256], 'spe_ln_bB': [128, 2, 256],
        'spe_in_pk': [2, 128, 2, 1160], 'spe_conv_pk': [128, 2, 5, 4],
        'spe_out_pk': [2, 128, 4, 256],
        'cprj_pk': [128, 5, 128],
        'aqT': [128, 128], 'akT': [128, 128], 'avT': [128, 128], 'aoT': [128, 128],
        'sqT': [128, 2, 256], 'skT': [128, 2, 256], 'svT': [128, 2, 256],
        'soT': [128, 2, 256], 'svbB': [128, 256], 'sobB': [128, 256],
        'dsw_pk': [128, 9, 128], 'ds_ln_wB': [64, 128], 'ds_ln_bB': [64, 128],
        'lnwb': [2, 3, 128], 'colpak': [128, len(COL_ORDER)],
    }
    w_t = {k: din(k, shp, F16 if k in F16_WEIGHTS else F32)
           for k, shp in wnames.items()}

    out = nc.dram_tensor("out", [BPC, 8, 8, C], F32, kind="ExternalOutput")
    tap_t = {}

    with tile.TileContext(nc) as tc:
        import contextlib
        stk = contextlib.ExitStack()
        sb = stk.enter_context(tc.tile_pool(name="sb", bufs=1))
        ps1 = stk.enter_context(tc.tile_pool(name="ps1", bufs=2, space="PSUM"))
        ps2 = stk.enter_context(tc.tile_pool(name="ps2", bufs=3, space="PSUM"))
        psS = stk.enter_context(tc.tile_pool(name="psS", bufs=2, space="PSUM"))
        psD = stk.enter_context(tc.tile_pool(name="psD", bufs=1, space="PSUM"))

        BUFS2 = {"cv_a0", "cv_a1", "rowA", "rowB", "tm_tmp", "ssd_Dt",
                 "perm_oh", "ssd_MT", "spa_xtm",
                 "spe_xtm", "spa_ygt", "spa_ynt", "spe_ygt",
                 "spe_y0", "spe_ynt", "spa_acumT",
                 "spe_acumT", "xc_0", "xc_1", "xc_2", "xc_3", "xc_B", "xc_C",
                 "cv_x2", "cv_x3",
                 "spe_h2sb", "x2f_tmp", "sp2_q2", "sp2_k2", "sp2_v2",
                 "sp2_a2", "sp2_a2T", "sp2_o2", "sp2_ex", "at_ex", "at_aw",
                 "mb_dtv", "mb_acum", "pball",
                 "spe_xn", "spe_u", "spe_xsn", "ds_cmp", "rowC", "ln_rstd",
                 "ln_out"}
        F16TAGS = {
            "ones4", "irow_f", "perm_oh", "tm_tmp", "ln_rhs", "ln_rstd",
            "ln_out", "w_spa_in", "w_spe_in", "w_spe_out", "mb_dtv", "rowC",
            "xc_B", "xc_C", "ssd_MT", "spa_xtm", "spe_xtm", "spa_ynt",
            "spe_ynt", "x2f_tmp", "at_ctr", "at_q", "at_qd", "at_K", "at_vo",
            "at_aw", "sp2_q2", "sp2_k2", "sp2_v2", "sp2_a2T", "sp2_o2",
            "ds_cmp", "c_identh", "x0", "mb_xp", "spe_xsn", "xs2", "sp2_a2",
            "spe_h2sb", "cv_x0", "cv_x1", "cv_x2", "cv_x3", "cv_B", "cv_C",
            "cv_a0", "cv_a1", "xc_0", "xc_1", "xc_2", "xc_3", "mb_zsil",
            "spa_ygt", "spe_ygt", "spa_y0t", "spe_y0", "sqy16", "xf16",
            "sq16", "ssd_Et", "ssd_m0m",
        }
        F16TAGS.update("w_" + k for k in F16_WEIGHTS)
        F16TAGS.update("c_" + k for k in ("E_spaJ", "E_speJ", "E_attn"))

        def T(shape, tag, dt=None):
            if dt is None:
                dt = F16 if tag in F16TAGS else F32
            return sb.tile(shape, dt, tag=tag, name=tag,
                           bufs=2 if tag in BUFS2 else 1)

        def P512(tag="b512"):
            return ps1.tile([128, 512], F32, tag=tag, name=tag)

        def P256(tag="b256"):
            return ps2.tile([128, 256], F32, tag=tag, name=tag)

        def tap(name, ap_fn):
            # ap_fn: callable giving (dram_shape, writer) – writer(dram) DMAs data
            if name in taps:
                shape, writer = ap_fn()
                t = nc.dram_tensor("t_" + name, shape, F32, kind="ExternalOutput")
                tap_t[name] = t
                writer(t)

        dma = nc.sync.dma_start
        V = nc.vector
        S = nc.scalar
        G = nc.gpsimd

        def MM(out, lhsT, rhs, **kw):
            return nc.tensor.matmul(out, lhsT, rhs, **kw)

        def TR(out, in_, identity, **kw):
            return nc.tensor.matmul(out, in_, identity, is_transpose=True, **kw)

        # ---------- load constants (stage0-critical first) ----------
        ct = {}

        def load_c(names):
            for k in names:
                if k in ct or k == 'onesrow':
                    continue
                ct[k] = T(list(cst[k].shape), "c_" + k)
                dma(ct[k][:], cst_t[k][:])

        load_c(['pe_fm', 'iotaC', 'ident', 'identh'])
        ones32 = T([128, 128], "ones32")
        V.memset(ones32[:], 1.0)
        onescol32 = ones32[:, 0:1]
        onesrow32 = ones32[0:1, :]

        # ---------- preload weights (staged: mamba weights now, attention
        # and downsample weights deferred until after stage0 issue order) ----
        wt = {}

        def load_w(names):
            for name in names:
                if name in wt or name in ('spa_in_wT', 'spe_in_pk',
                                          'spe_out_pk'):
                    continue
                t = T(wnames[name], "w_" + name)
                dma(t[:], w_t[name][:])
                wt[name] = t

        load_c(list(cst.keys()))
        load_w(['colpak', 'lnwb', 'spa_conv_pk', 'spa_out_pk'])
        colpak = wt['colpak']

        def col(key, p=128):
            return colpak[0:p, CIDX[key]:CIDX[key] + 1]

        ones4 = T([128, 128], "ones4")
        V.memset(ones4[:], 1.0)
        epscol = T([128, 1], "epscol")
        V.memset(epscol[:], EPS)
        onescol = ones4[:, 0:1]       # [128,1]
        onesrow1 = ones4[0:1, :]      # [1,128]
        ident = ct['ident']

        # ---------- stage 0: embed + permute ----------
        xb = T([128, BPC, L], "xb")
        for s in range(BPC):
            dma(xb[:, s, :], x2[s])
        x0 = T([128, BPC, L], "x0")
        V.tensor_tensor(
            x0[:], xb[:],
            ct['pe_fm'][:].unsqueeze(1).to_broadcast((128, BPC, L)),
            op=ALU.add)

        idxr = T([1, BPC, L], "irow_raw", I32)
        dma(idxr[:], idx[None, :, :])
        idxf = T([1, BPC, L], "irow_f")
        V.tensor_copy(idxf[:], idxr[:])

        xs = T([128, BPC, L], "xs")
        for s in range(BPC):
            # PmT[st][sp][t] = (idx[t] == st*128+sp)
            idxB = P512()
            MM(idxB[:, 0:L], onesrow1, idxf[:, s, :], start=True, stop=True)
            PmT = T([128, 2, L], "perm_oh")
            for st in range(2):
                V.tensor_scalar(PmT[:, st, :], idxB[:, 0:L],
                                ct['iotaC'][:, st:st + 1], None,
                                op0=ALU.is_equal)
            # x0 token-major
            x0tm = T([128, 2, 128], "tm_tmp")
            for tt in range(2):
                ptr = P256()
                ptr16 = ptr[:].bitcast(F16)
                TR(ptr16[:, 0:128], x0[:, s, tt * 128:(tt + 1) * 128],
                   ct['identh'][:])
                S.copy(x0tm[:, tt, :], ptr16[:, 0:128])
            pxs = P256()
            for st in range(2):
                MM(pxs[:], x0tm[:, st, :], PmT[:, st, :],
                                 start=(st == 0), stop=(st == 1))
            S.copy(xs[:, s, :], pxs[:])

        def tap_batched(t_sb, shape_per_s):
            def writer(dram):
                for s in range(BPC):
                    dma(dram[s], t_sb[:, s, :])
            return ([BPC] + shape_per_s, writer)

        tap("xs0", lambda: tap_batched(xs, [128, L]))

        load_w(['spe_ln_wB', 'spe_ln_bB', 'spe_conv_pk'])

        # ================= shared helpers =================
        lnrhs = T([2, 512], "ln_rhs")
        dma(lnrhs[1:2, :], cst_t['onesrow'][None, :])

        def part_ln(xflat, lnidx):
            """LayerNorm over the channel (partition) dim of [128, 512]."""
            xf16 = T([128, 512], "xf16")
            S.copy(xf16[:], xflat)
            sq = T([128, 512], "sq16")
            S.activation(sq[:], xf16[:], AF.Square)
            msum = psS.tile([1, 512], F32, tag="small", name="small")
            MM(msum[:], ones4[:, 0:1], xf16[:], start=True, stop=True)
            murow = T([1, 512], "ln_mu")
            V.tensor_scalar(murow[:], msum[:], 1.0 / 128, None, op0=ALU.mult)
            ssum = psS.tile([1, 512], F32, tag="small", name="small")
            MM(ssum[:], ones4[:, 0:1], sq[:], start=True, stop=True)
            mu2 = T([1, 512], "rowA")
            V.tensor_mul(mu2[:], murow[:], murow[:])
            var = T([1, 512], "rowB")
            V.scalar_tensor_tensor(var[:], ssum[:], 1.0 / 128, mu2[:],
                                   op0=ALU.mult, op1=ALU.subtract)
            lnv = T([1, 512], "rowA")
            S.activation(lnv[:], var[:], AF.Ln, bias=epscol[0:1, 0:1])
            rstd = T([1, 512], "ln_rstd")
            S.activation(rstd[:], lnv[:], AF.Exp, scale=-0.5)
            V.scalar_tensor_tensor(lnrhs[0:1, :], murow[:], -1.0, rstd[:],
                                   op0=ALU.mult, op1=ALU.mult)
            Rp = P512()
            MM(Rp[:], wt['lnwb'][:, lnidx, :], lnrhs[:],
                             start=True, stop=True)
            rstdB = P512()
            MM(rstdB[:], onesrow1, rstd[:], start=True, stop=True)
            wcol = col(("lnw_spa0", "lnw_spa1", "lnw_norm")[lnidx])
            tmp = T([128, 512], "ln_tmp")
            V.tensor_mul(tmp[:], xflat, rstdB[:])
            xln = T([128, 512], "ln_out")
            V.scalar_tensor_tensor(xln[:], tmp[:], wcol, Rp[:],
                                   op0=ALU.mult, op1=ALU.add)
            return xln

        def convchain(buf, wc, cb, P, W, tag, E=None):
            """Causal depthwise conv (k=4) + silu. buf [P, 2, W+3]; returns [P, 2, W]."""
            E = E or V
            a0 = T([P, 2, W], "cv_a0")
            E.tensor_scalar(a0[:], buf[:, :, 0:W], wc[:, 0:1], None, op0=ALU.mult)
            a1 = T([P, 2, W], "cv_a1")
            E.scalar_tensor_tensor(a1[:], buf[:, :, 1:W + 1], wc[:, 1:2], a0[:],
                                   op0=ALU.mult, op1=ALU.add)
            a2 = T([P, 2, W], "cv_a0")
            E.scalar_tensor_tensor(a2[:], buf[:, :, 2:W + 2], wc[:, 2:3], a1[:],
                                   op0=ALU.mult, op1=ALU.add)
            a3 = T([P, 2, W], "cv_a1")
            E.scalar_tensor_tensor(a3[:], buf[:, :, 3:W + 3], wc[:, 3:4], a2[:],
                                   op0=ALU.mult, op1=ALU.add)
            xc = T([P, 2, W], tag)
            S.activation(xc[:], a3[:], AF.Silu, bias=cb[:, 0:1])
            return xc

        # ================= spa mamba =================
        def spa_mamba(i, xs):
            xflat = xs[:].rearrange("p s t -> p (s t)")
            xln = part_ln(xflat, i)
            tap(f"xln{i}", lambda: ([128, 512], lambda d: dma(d[:], xln[:])))
            inw_t = T([128, 644], "w_spa_in")
            dma(inw_t[:], w_t['spa_in_wT'][i])
            inw = inw_t[:]
            # dt chain first: keeps scalar engine in the ln/exp table while
            # part_ln's exp is still resident, before the silu block
            pdt = psS.tile([4, 512], F32, tag="small", name="small")
            MM(pdt[:], inw[:, 640:644], xln[:], start=True, stop=True)
            e1 = T([4, 512], "rowA")
            S.activation(e1[:], pdt[:], AF.Exp, bias=col(f"spa_dtb{i}", 4))
            # softplus via ln(1+u) Taylor (|u|<0.5): keeps scalar engine out
            # of the Ln table mid-silu-run
            u2 = T([4, 512], "rowB")
            V.tensor_mul(u2[:], e1[:], e1[:])
            u3 = T([4, 512], "tay3")
            V.tensor_mul(u3[:], u2[:], e1[:])
            u4 = T([4, 512], "tay4")
            V.tensor_mul(u4[:], u2[:], u2[:])
            u5 = T([4, 512], "tay5")
            V.tensor_mul(u5[:], u2[:], u3[:])
            d1 = T([4, 512], "tay6")
            V.scalar_tensor_tensor(d1[:], u2[:], -0.5, e1[:],
                                   op0=ALU.mult, op1=ALU.add)
            d2 = T([4, 512], "rowB")
            V.scalar_tensor_tensor(d2[:], u3[:], 1.0 / 3, d1[:],
                                   op0=ALU.mult, op1=ALU.add)
            d3 = T([4, 512], "tay3")
            V.scalar_tensor_tensor(d3[:], u4[:], -0.25, d2[:],
                                   op0=ALU.mult, op1=ALU.add)
            dtv = T([4, 512], "mb_dtv")
            V.scalar_tensor_tensor(dtv[:], u5[:], 0.2, d3[:],
                                   op0=ALU.mult, op1=ALU.add)
            dtA = T([4, 512], "rowA")
            V.tensor_scalar(dtA[:], dtv[:], col(f"spa_alog{i}", 4), -1.0,
                            op0=ALU.mult, op1=ALU.mult)
            acum = T([4, 512], "mb_acum")
            aflat = T([1, 2, 1024], "aflat")
            for s in range(BPC):
                V.tensor_tensor_scan(acum[:, s * 256:(s + 1) * 256],
                                     dtA[:, s * 256:(s + 1) * 256],
                                     dtA[:, s * 256:(s + 1) * 256], 0.0,
                                     op0=ALU.add, op1=ALU.bypass)
                dma(aflat[0:1, s, :].rearrange("o (p f) -> o p f", p=4),
                    acum[:, s * 256:(s + 1) * 256])
            # in_proj: z (2 blocks), x (2 blocks), B, C
            zsil = T([128, 2, 512], "mb_zsil")
            for j in range(2):
                pz = P512()
                MM(pz[:], inw[:, j * 128:(j + 1) * 128], xln[:],
                                 start=True, stop=True)
                S.activation(zsil[:, j, :], pz[:], AF.Silu)
            cvx = []
            for j in range(2):
                px = P512()
                MM(px[:], inw[:, 256 + j * 128:256 + (j + 1) * 128], xln[:],
                                 start=True, stop=True)
                buf = T([128, 2, 259], f"cv_x{j}")
                V.memset(buf[:, :, 0:3], 0.0)
                S.copy(buf[:, :, 3:259], px[:].rearrange("p (s t) -> p s t", s=2))
                cvx.append(buf)
            pbc = P512()
            MM(pbc[:], inw[:, 512:640], xln[:], start=True, stop=True)
            bufbc = T([128, 2, 259], "cv_B")
            V.memset(bufbc[:, :, 0:3], 0.0)
            S.copy(bufbc[:, :, 3:259], pbc[:].rearrange("p (s t) -> p s t", s=2))
            tap(f"dtv{i}", lambda: ([4, 512], lambda d: dma(d[:], dtv[:])))
            tap(f"acum{i}", lambda: ([4, 512], lambda d: dma(d[:], acum[:])))
            # conv + silu
            xc = []
            for j in range(2):
                xc.append(convchain(cvx[j], wt['spa_conv_pk'][:, i, j, :],
                                    col(f"spa_cb{i}_{j}"), 128, 256, f"xc_{j}"))
            xcBC = convchain(bufbc, wt['spa_conv_pk'][:, i, 2, :],
                             col(f"spa_cbBC{i}"), 128, 256, "xc_B")
            xcB = xcBC[0:64]
            xcC = T([64, 2, 256], "xc_C")
            dma(xcC[:], xcBC[64:128])
            if i == 0:
                tap("xc00", lambda: ([128, 512], lambda d: dma(
                    d[:], xc[0][:].rearrange("p s t -> p (s t)"))))
                tap("xcB0", lambda: ([64, 512], lambda d: dma(
                    d[:], xcB[:].rearrange("p s t -> p (s t)"))))
                tap("xcC0", lambda: ([64, 512], lambda d: dma(
                    d[:], xcC[:].rearrange("p s t -> p (s t)"))))
            # dt-scaled x (feature-major): xp[:, j, :] = xc[j] * dtB_j
            xp = T([128, 2, 512], "mb_xp")
            for j in range(2):
                pdb = P512()
                MM(pdb[:], ct['E_spaJ'][:, j, :], dtv[:], start=True, stop=True)
                V.tensor_mul(xp[:, j, :],
                             xc[j][:].rearrange("p s t -> p (s t)"), pdb[:])
            h1 = T([128, 2, 256], "h1")
            for s in range(BPC):
                # token-major dt-scaled x: xtm [t(128), st, hp(256)]
                xtm = T([128, 2, 256], "spa_xtm")
                for st in range(2):
                    for j in range(2):
                        ptr = P256()
                        ptr16 = ptr[:].bitcast(F16)
                        TR(
                            ptr16[:, 0:128],
                            xp[:, j, s * 256 + st * 128: s * 256 + (st + 1) * 128],
                            ct['identh'][:])
                        S.copy(xtm[:, st, j * 128:(j + 1) * 128], ptr16[:, 0:128])
                # masked M0^T per s-tile
                m0m = T([128, 2, 256], "ssd_m0m")
                for st in range(2):
                    pm0 = P256()
                    MM(pm0[:], xcB[:, s, st * 128:(st + 1) * 128],
                                     xcC[:, s, :], start=True, stop=True)
                    V.tensor_mul(m0m[:, st, :], pm0[:], ct['maskT_spa'][:, st, :])
                # Acum transposes + strided copy
                acumT = T([128, 2, 4], "spa_acumT")
                for tt in range(2):
                    ptr = P256()
                    TR(ptr[:, 0:4],
                                        acum[:, s * 256 + tt * 128: s * 256 + (tt + 1) * 128],
                                        ident[0:4, 0:4])
                    S.copy(acumT[:, tt, :], ptr[:, 0:4])
                pball = P512()
                MM(pball[:], ones32[0:1, :], aflat[:, s, 0:512],
                   start=True, stop=True)
                pbal2 = P512()
                MM(pbal2[:], ones32[0:1, :], aflat[:, s, 512:1024],
                   start=True, stop=True)
                # Y accumulation per head over s-tiles
                ypsl = [P256(), P256()]
                for st in range(2):
                    Dt = T([128, 4, 256], "ssd_Dt")
                    for h in range(H1):
                        pbx = pball if h < 2 else pbal2
                        V.tensor_scalar(Dt[:, h, :],
                                        pbx[:, (h % 2) * 256:(h % 2 + 1) * 256],
                                        acumT[:, st, h:h + 1], 0.0,
                                        op0=ALU.subtract, op1=ALU.min)
                    Et = T([128, 4, 256], "ssd_Et")
                    S.activation(Et[:].rearrange("p h t -> p (h t)"),
                                 Dt[:].rearrange("p h t -> p (h t)"), AF.Exp)
                    MT = T([128, 4, 256], "ssd_MT")
                    V.tensor_tensor(MT[:], Et[:],
                                    m0m[:, st, :].unsqueeze(1).to_broadcast((128, 4, 256)),
                                    op=ALU.mult)
                    if i == 0 and s == 0 and st == 0:
                        tap("Dt00", lambda: ([128, 1024], lambda d: dma(
                            d[:], Dt[:].rearrange("p h t -> p (h t)"))))
                        tap("MT00", lambda: ([128, 1024], lambda d: dma(
                            d[:], MT[:].rearrange("p h t -> p (h t)"))))
                    for h in range(H1):
                        MM(ypsl[h // 2][(h % 2) * 64:(h % 2) * 64 + 64, :],
                                         xtm[:, st, h * 64:(h + 1) * 64],
                                         MT[:, h, :],
                                         start=(st == 0), stop=(st == 1),
                                         tile_position=(0, (h % 2) * 64),
                                         skip_group_check=True)
                if i == 0 and s == 0:
                    tap("xtm0", lambda: ([128, 512], lambda d: dma(
                        d[:], xtm[:].rearrange("p s t -> p (s t)"))))
                    tap("m0m0", lambda: ([128, 512], lambda d: dma(
                        d[:], m0m[:].rearrange("p s t -> p (s t)"))))
                    tap("acumT0", lambda: ([128, 8], lambda d: dma(
                        d[:], acumT[:].rearrange("p s t -> p (s t)"))))
                    tap("acs0", lambda: ([128, 256], lambda d: dma(d[:], acs[:])))
                ygt = T([128, 2, 256], "spa_ygt")
                y0t = T([128, 2, 256], "spa_y0t")
                for j in range(2):
                    V.scalar_tensor_tensor(y0t[:, j, :], xc[j][:, s, :],
                                           col(f"spa_dpc{i}_{j}"),
                                           ypsl[j][:], op0=ALU.mult, op1=ALU.add)
                    V.tensor_mul(ygt[:, j, :], y0t[:, j, :],
                                 zsil[:, j, s * 256:(s + 1) * 256])
                if i == 0 and s == 0:
                    tap("y00", lambda: ([128, 512], lambda d: dma(
                        d[:], y0t[:].rearrange("p j t -> p (j t)"))))
                    tap("zsil0", lambda: ([128, 1024], lambda d: dma(
                        d[:], zsil[:].rearrange("p j t -> p (j t)"))))
                # gated RMS norm over d_inner
                sqy = T([128, 2, 256], "sqy16")
                S.activation(sqy[:].rearrange("p j t -> p (j t)"),
                             ygt[:].rearrange("p j t -> p (j t)"), AF.Square)
                ssy = psS.tile([1, 256], F32, tag="small", name="small")
                for j in range(2):
                    MM(ssy[:], ones4[:, 0:1], sqy[:, j, :],
                                     start=(j == 0), stop=(j == 1))
                rl = T([1, 256], "rowB")
                S.activation(rl[:], ssy[:], AF.Ln, bias=epscol[0:1, 0:1],
                             scale=1.0 / 256)
                rrow = T([1, 256], "rowC")
                S.activation(rrow[:], rl[:], AF.Exp, scale=-0.5)
                rB = P256()
                MM(rB[:], onesrow1, rrow[:], start=True, stop=True)
                ynt = T([128, 2, 256], "spa_ynt")
                for j in range(2):
                    V.scalar_tensor_tensor(ynt[:, j, :], ygt[:, j, :],
                                           col(f"spa_rwc{i}_{j}"),
                                           rB[:], op0=ALU.mult, op1=ALU.mult)
                if i == 0 and s == 0:
                    tap("ygt0", lambda: ([128, 512], lambda d: dma(
                        d[:], ygt[:].rearrange("p s t -> p (s t)"))))
                    tap("ynt0", lambda: ([128, 512], lambda d: dma(
                        d[:], ynt[:].rearrange("p s t -> p (s t)"))))
                pop = P256()
                for j in range(2):
                    MM(pop[:], wt['spa_out_pk'][:, i, j, :], ynt[:, j, :],
                                     start=(j == 0), stop=(j == 1))
                V.tensor_add(h1[:, s, :], pop[:], xs[:, s, :])
            return h1

        # ================= spe mamba =================
        def spe_mamba(i, h1):
            # LayerNorm over the 256 features (free dim), batched samples
            mus = T([128, 2], "spe_mus")
            V.tensor_reduce(mus[:], h1[:], axis=AX.X, op=ALU.add)
            sq2 = T([128, 512], "sq_tmp")
            S.activation(sq2[:], h1[:].rearrange("p s t -> p (s t)"), AF.Square)
            ss2 = T([128, 2], "spe_ss2")
            V.tensor_reduce(ss2[:], sq2[:].rearrange("p (s t) -> p s t", s=2),
                            axis=AX.X, op=ALU.add)
            mean = T([128, 2], "spe_mean")
            V.tensor_scalar(mean[:], mus[:], 1.0 / 256, None, op0=ALU.mult)
            m2 = T([128, 2], "spe_m2")
            V.tensor_mul(m2[:], mean[:], mean[:])
            var2 = T([128, 2], "spe_var")
            V.scalar_tensor_tensor(var2[:], ss2[:], 1.0 / 256, m2[:],
                                   op0=ALU.mult, op1=ALU.subtract)
            l2t = T([128, 2], "spe_l2")
            S.activation(l2t[:], var2[:], AF.Ln, bias=epscol[:, 0:1])
            rstd2 = T([128, 2], "spe_rstd")
            S.activation(rstd2[:], l2t[:], AF.Exp, scale=-0.5)
            X2f = T([128, 2, 2, 128], "x2f_tmp")
            for s in range(BPC):
                xn = T([128, 256], "spe_xn")
                V.tensor_scalar(xn[:], h1[:, s, :], mean[:, s:s + 1], rstd2[:, s:s + 1],
                                op0=ALU.subtract, op1=ALU.mult)
                u = T([128, 256], "spe_u")
                V.tensor_mul(u[:], xn[:], wt['spe_ln_wB'][:, i, :])
                xsn = T([128, 256], "spe_xsn")
                V.tensor_add(xsn[:], u[:], wt['spe_ln_bB'][:, i, :])
                for ft in range(2):
                    ptr = P256()
                    ptr16 = ptr[:].bitcast(F16)
                    TR(ptr16[:, 0:128], xsn[:, ft * 128:(ft + 1) * 128],
                       ct['identh'][:])
                    S.copy(X2f[:, s, ft, :], ptr16[:, 0:128])
            # in_proj (samples batched along free): out cols ordered (s, t2)
            inw2t = T([128, 2, 1160], "w_spe_in")
            dma(inw2t[:], w_t['spe_in_pk'][i])
            inw2 = inw2t[:]
            ow2t = T([128, 4, 256], "w_spe_out")
            dma(ow2t[:], w_t['spe_out_pk'][i])
            ow2 = ow2t[:]

            def mm2(out_ap, off, width):
                for k in range(2):
                    MM(out_ap,
                                     inw2[:, k, off:off + width],
                                     X2f[:, :, k, :],
                                     start=(k == 0), stop=(k == 1))
            pdt = psS.tile([8, 256], F32, tag="small", name="small")
            for k in range(2):
                MM(pdt[:], inw2[:, k, 1152:1160],
                                 X2f[:, :, k, :], start=(k == 0), stop=(k == 1))
            e1 = T([8, 256], "rowA")
            S.activation(e1[:], pdt[:], AF.Exp, bias=col(f"spe_dtb{i}", 8))
            u2 = T([8, 256], "rowB")
            V.tensor_mul(u2[:], e1[:], e1[:])
            u3 = T([8, 256], "tay3")
            V.tensor_mul(u3[:], u2[:], e1[:])
            u4 = T([8, 256], "tay4")
            V.tensor_mul(u4[:], u2[:], u2[:])
            u5 = T([8, 256], "tay5")
            V.tensor_mul(u5[:], u2[:], u3[:])
            d1 = T([8, 256], "tay6")
            V.scalar_tensor_tensor(d1[:], u2[:], -0.5, e1[:],
                                   op0=ALU.mult, op1=ALU.add)
            d2 = T([8, 256], "rowB")
            V.scalar_tensor_tensor(d2[:], u3[:], 1.0 / 3, d1[:],
                                   op0=ALU.mult, op1=ALU.add)
            d3 = T([8, 256], "tay3")
            V.scalar_tensor_tensor(d3[:], u4[:], -0.25, d2[:],
                                   op0=ALU.mult, op1=ALU.add)
            dtv = T([8, 256], "mb_dtv")
            V.scalar_tensor_tensor(dtv[:], u5[:], 0.2, d3[:],
                                   op0=ALU.mult, op1=ALU.add)
            dtA = T([8, 256], "rowA")
            V.tensor_scalar(dtA[:], dtv[:], col(f"spe_alog{i}", 8), -1.0,
                            op0=ALU.mult, op1=ALU.mult)
            acum = T([8, 256], "mb_acum")
            aflat = T([1, 2, 1024], "aflat")
            for s in range(BPC):
                V.tensor_tensor_scan(acum[:, s * 128:(s + 1) * 128],
                                     dtA[:, s * 128:(s + 1) * 128],
                                     dtA[:, s * 128:(s + 1) * 128], 0.0,
                                     op0=ALU.add, op1=ALU.bypass)
                dma(aflat[0:1, s, :].rearrange("o (p f) -> o p f", p=8),
                    acum[:, s * 128:(s + 1) * 128])
            z2sil = T([128, 4, 256], "mb_zsil")
            for j in range(4):
                pz = P256()
                mm2(pz[:], j * 128, 128)
                S.activation(z2sil[:, j, :], pz[:], AF.Silu)
            cvx2 = []
            for j in range(4):
                px = P256()
                mm2(px[:], 512 + j * 128, 128)
                buf = T([128, 2, 131], f"cv_x{j}")
                V.memset(buf[:, :, 0:3], 0.0)
                S.copy(buf[:, :, 3:131], px[:].rearrange("p (s t) -> p s t", s=2))
                cvx2.append(buf)
            pbc = P256()
            for k in range(2):
                MM(pbc[:], inw2[:, k, 1024:1152],
                   X2f[:, :, k, :], start=(k == 0), stop=(k == 1))
            bufbc = T([128, 2, 131], "cv_B")
            V.memset(bufbc[:, :, 0:3], 0.0)
            S.copy(bufbc[:, :, 3:131], pbc[:].rearrange("p (s t) -> p s t", s=2))
            # conv + silu
            xc2 = []
            for j in range(4):
                xc2.append(convchain(cvx2[j], wt['spe_conv_pk'][:, i, j, :],
                                     col(f"spe_cb{i}_{j}"), 128, 128, f"xc_{j}"))
            xcBC = convchain(bufbc, wt['spe_conv_pk'][:, i, 4, :],
                             col(f"spe_cbBC{i}"), 128, 128, "xc_B")
            xcB = xcBC[0:64]
            xcC = T([64, 2, 128], "xc_C")
            dma(xcC[:], xcBC[64:128])
            # dt-scaled x
            xp2 = T([128, 4, 256], "mb_xp")
            for j in range(4):
                pdb = P256()
                MM(pdb[:], ct['E_speJ'][:, j, :], dtv[:], start=True, stop=True)
                V.tensor_mul(xp2[:, j, :],
                             xc2[j][:].rearrange("p s t -> p (s t)"), pdb[:])
            xs_new = T([128, 2, 256], "xs")
            for s in range(BPC):
                xtm2 = T([128, 512], "spe_xtm")
                for j in range(4):
                    ptr = P256()
                    ptr16 = ptr[:].bitcast(F16)
                    TR(ptr16[:, 0:128],
                       xp2[:, j, s * 128:(s + 1) * 128], ct['identh'][:])
                    S.copy(xtm2[:, j * 128:(j + 1) * 128], ptr16[:, 0:128])
                m0m2 = T([128, 128], "ssd_m0m")
                pm0 = P256()
                MM(pm0[:, 0:128], xcB[:, s, :], xcC[:, s, :],
                                 start=True, stop=True)
                V.tensor_mul(m0m2[:], pm0[:, 0:128], ct['maskT_spe'][:])
                acumT = T([128, 8], "spe_acumT")
                ptr = P256()
                TR(ptr[:, 0:8], acum[:, s * 128:(s + 1) * 128],
                                    ident[0:8, 0:8])
                S.copy(acumT[:], ptr[:, 0:8])
                pball = P512()
                MM(pball[:], ones32[0:1, :], aflat[:, s, 0:512],
                   start=True, stop=True)
                pbal2 = P512()
                MM(pbal2[:], ones32[0:1, :], aflat[:, s, 512:1024],
                   start=True, stop=True)
                Dt = T([128, 8, 128], "ssd_Dt")
                for h in range(H2):
                    pbx = pball if h < 4 else pbal2
                    V.tensor_scalar(Dt[:, h, :],
                                    pbx[:, (h % 4) * 128:(h % 4 + 1) * 128],
                                    acumT[:, h:h + 1], 0.0,
                                    op0=ALU.subtract, op1=ALU.min)
                Et = T([128, 8, 128], "ssd_Et")
                S.activation(Et[:].rearrange("p h t -> p (h t)"),
                             Dt[:].rearrange("p h t -> p (h t)"), AF.Exp)
                MT = T([128, 8, 128], "ssd_MT")
                V.tensor_tensor(MT[:], Et[:],
                                m0m2[:].unsqueeze(1).to_broadcast((128, 8, 128)),
                                op=ALU.mult)
                ygt2 = T([128, 4, 128], "spe_ygt")
                for j in range(4):
                    yp = P256()
                    for hh in range(2):
                        h = 2 * j + hh
                        MM(yp[hh * 64:hh * 64 + 64, 0:128],
                                         xtm2[:, h * 64:(h + 1) * 64],
                                         MT[:, h, :], start=True, stop=True,
                                         tile_position=(0, hh * 64),
                                         skip_group_check=True)
                    y0 = T([128, 128], "spe_y0")
                    V.scalar_tensor_tensor(y0[:], xc2[j][:, s, :],
                                           col(f"spe_dpc{i}_{j}"),
                                           yp[:, 0:128], op0=ALU.mult, op1=ALU.add)
                    V.tensor_mul(ygt2[:, j, :], y0[:],
                                 z2sil[:, j, s * 128:(s + 1) * 128])
                sqy = T([128, 4, 128], "sqy16")
                S.activation(sqy[:].rearrange("p j t -> p (j t)"),
                             ygt2[:].rearrange("p j t -> p (j t)"), AF.Square)
                ssy = psS.tile([1, 128], F32, tag="small", name="small")
                for j in range(4):
                    MM(ssy[:], ones4[:, 0:1], sqy[:, j, :],
                                     start=(j == 0), stop=(j == 3))
                rl = T([1, 128], "rowB")
                S.activation(rl[:], ssy[:], AF.Ln, bias=epscol[0:1, 0:1],
                             scale=1.0 / 512)
                rrow = T([1, 128], "rowC")
                S.activation(rrow[:], rl[:], AF.Exp, scale=-0.5)
                rB = P256()
                MM(rB[:, 0:128], onesrow1, rrow[:], start=True, stop=True)
                ynt = T([128, 4, 128], "spe_ynt")
                for j in range(4):
                    V.scalar_tensor_tensor(ynt[:, j, :], ygt2[:, j, :],
                                           col(f"spe_rwc{i}_{j}"),
                                           rB[:, 0:128], op0=ALU.mult, op1=ALU.mult)
                for ft in range(2):
                    ph2 = P256()
                    for k in range(4):
                        MM(ph2[:, 0:128],
                                         ow2[:, k, ft * 128:(ft + 1) * 128],
                                         ynt[:, k, :], start=(k == 0), stop=(k == 3))
                    h2sb = T([128, 128], "spe_h2sb")
                    S.copy(h2sb[:], ph2[:, 0:128])
                    ptr = P256()
                    ptr16 = ptr[:].bitcast(F16)
                    TR(ptr16[:, 0:128], h2sb[:], ct['identh'][:])
                    V.tensor_add(xs_new[:, s, ft * 128:(ft + 1) * 128],
                                 ptr16[:, 0:128],
                                 h1[:, s, ft * 128:(ft + 1) * 128])
            return xs_new

        # ================= layers =================
        cur = xs
        for i in range(2):
            h1 = spa_mamba(i, cur)
            tap(f"h1_{i}", lambda: tap_batched(h1, [128, L]))
            cur = spe_mamba(i, h1)
            tap(f"xsl{i + 1}", lambda: tap_batched(cur, [128, L]))

        load_w(['cprj_pk', 'aqT', 'akT', 'avT', 'aoT', 'sqT', 'skT', 'svT',
                'soT', 'svbB', 'sobB', 'dsw_pk', 'ds_ln_wB', 'ds_ln_bB'])

        # ================= final LN =================
        xfl = part_ln(cur[:].rearrange("p s t -> p (s t)"), 2)
        xf = xfl[:].rearrange("p (s t) -> p s t", s=BPC)
        tap("xf", lambda: ([BPC, 128, L],
                           lambda d: [dma(d[s], xf[:, s, :]) for s in range(BPC)]))

        # ================= spa attention (center query) =================
        pctr = psS.tile([128, 2], F32, tag="small", name="small")
        for l in range(5):
            MM(pctr[:], wt['cprj_pk'][:, l, :], xf[:, :, l],
                             start=(l == 0), stop=(l == 4))
        ctr = T([128, 2], "at_ctr")
        S.activation(ctr[:], pctr[:], AF.Identity, bias=col("cprj_b"))
        pq = psS.tile([128, 2], F32, tag="small", name="small")
        MM(pq[:], wt['aqT'][:], ctr[:], start=True, stop=True)
        qsb = T([128, 2], "at_q")
        S.activation(qsb[:], pq[:], AF.Identity, bias=col("aq_b"))
        pk = P512()
        MM(pk[:], wt['akT'][:], xfl[:], start=True, stop=True)
        Ksb = T([128, 2, 256], "at_K")
        S.activation(Ksb[:].rearrange("p s t -> p (s t)"), pk[:], AF.Identity,
                     bias=col("ak_b"))
        pv = P512()
        MM(pv[:], wt['avT'][:], xfl[:], start=True, stop=True)
        Vsb = T([128, 2, 256], "at_V")
        S.activation(Vsb[:].rearrange("p s t -> p (s t)"), pv[:], AF.Identity,
                     bias=col("av_b"))
        vo = T([128, 2, 256], "at_vo")
        plg = psS.tile([8, 2, 256], F32, tag="small", name="small")
        for s in range(BPC):
            qd = T([128, 8], "at_qd")
            V.tensor_tensor(qd[:], qsb[:, s:s + 1].to_broadcast((128, 8)),
                            ct['Emask_q'][:], op=ALU.mult)
            MM(plg[:, s, :], qd[:], Ksb[:, s, :], start=True, stop=True,
               skip_group_check=True)
        nm = T([8, 2], "at_nm")
        V.tensor_reduce(nm[:], plg[:], axis=AX.X, op=ALU.max, negate=True)
        sub = T([8, 2, 256], "at_ex")
        V.tensor_tensor(sub[:], plg[:],
                        nm[:].unsqueeze(2).to_broadcast((8, 2, 256)),
                        op=ALU.add)
        ex = T([8, 2, 256], "at_aw")
        S.activation(ex[:].rearrange("p s t -> p (s t)"),
                     sub[:].rearrange("p s t -> p (s t)"), AF.Exp, scale=0.25)
        sm = T([8, 2], "at_sm")
        V.tensor_reduce(sm[:], ex[:], axis=AX.X, op=ALU.add)
        rc = T([8, 2], "at_rc")
        V.reciprocal(rc[:], sm[:])
        aw = T([8, 2, 256], "at_aw", F16)
        V.tensor_tensor(aw[:], ex[:],
                        rc[:].unsqueeze(2).to_broadcast((8, 2, 256)),
                        op=ALU.mult)
        patB = P512()
        MM(patB[:], ct['E_attn'][:], aw[:].rearrange("p s t -> p (s t)"),
           start=True, stop=True)
        V.tensor_mul(vo[:].rearrange("p s t -> p (s t)"),
                     Vsb[:].rearrange("p s t -> p (s t)"), patB[:])
        pao = P512()
        MM(pao[:], wt['aoT'][:], vo[:].rearrange("p s t -> p (s t)"),
                         start=True, stop=True)
        xa = T([128, 2, 256], "xa")
        V.scalar_tensor_tensor(xa[:].rearrange("p s t -> p (s t)"), pao[:],
                               col("ao_b"), xfl[:], op0=ALU.add, op1=ALU.add)
        tap("xa", lambda: tap_batched(xa, [128, L]))

        # ================= spe attention =================
        X2a = T([128, 2, 2, 128], "x2f_tmp")
        for s in range(BPC):
            for ft in range(2):
                ptr = P256()
                TR(ptr[:, 0:128], xa[:, s, ft * 128:(ft + 1) * 128],
                                    ident[:])
                S.copy(X2a[:, s, ft, :], ptr[:, 0:128])
        q2 = T([128, 2, 2, 128], "sp2_q2")
        k2 = T([128, 2, 2, 128], "sp2_k2")
        for ot in range(2):
            pq2 = P256()
            for ft in range(2):
                MM(pq2[:], wt['sqT'][:, ft, ot * 128:(ot + 1) * 128],
                   X2a[:, :, ft, :], start=(ft == 0), stop=(ft == 1))
            for s in range(BPC):
                S.activation(q2[:, s, ot, :], pq2[:, s * 128:(s + 1) * 128],
                             AF.Identity, bias=col(f"sq_b{ot}"))
            pk2 = P256()
            for ft in range(2):
                MM(pk2[:], wt['skT'][:, ft, ot * 128:(ot + 1) * 128],
                   X2a[:, :, ft, :], start=(ft == 0), stop=(ft == 1))
            for s in range(BPC):
                S.activation(k2[:, s, ot, :], pk2[:, s * 128:(s + 1) * 128],
                             AF.Identity, bias=col(f"sk_b{ot}"))
        xs2 = T([128, 2, 256], "xs2")
        pa2 = ps1.tile([128, 512], F32, tag="b512", name="b512")
        for s in range(BPC):
            for ot in range(2):
                MM(pa2[:, s * 128:(s + 1) * 128], q2[:, s, ot, :],
                   k2[:, s, ot, :], start=(ot == 0), stop=(ot == 1),
                   skip_group_check=True)
        pa2v = pa2[:, 0:256].rearrange("p (s t) -> p s t", s=2)
        nm = T([128, 2], "sp2_nm")
        V.tensor_reduce(nm[:], pa2v, axis=AX.X, op=ALU.max, negate=True)
        sub2 = T([128, 2, 128], "sp2_ex")
        V.tensor_tensor(sub2[:], pa2v,
                        nm[:].unsqueeze(2).to_broadcast((128, 2, 128)),
                        op=ALU.add)
        ex = T([128, 2, 128], "sp2_sub")
        S.activation(ex[:].rearrange("p s t -> p (s t)"),
                     sub2[:].rearrange("p s t -> p (s t)"), AF.Exp,
                     scale=1.0 / 16)
        sm = T([128, 2], "sp2_sm")
        V.tensor_reduce(sm[:], ex[:], axis=AX.X, op=ALU.add)
        rc = T([128, 2], "sp2_rc")
        V.reciprocal(rc[:], sm[:])
        a2 = T([128, 2, 128], "sp2_a2")
        V.tensor_tensor(a2[:], ex[:],
                        rc[:].unsqueeze(2).to_broadcast((128, 2, 128)),
                        op=ALU.mult)
        for s in range(BPC):
            pv2 = P256()
            for ft in range(2):
                MM(pv2[:], X2a[:, s, ft, :], wt['svT'][:, ft, :],
                                 start=(ft == 0), stop=(ft == 1))
            v2 = T([128, 256], "sp2_v2")
            V.tensor_add(v2[:], pv2[:], wt['svbB'][:])
            pa2T = P256()
            pa2T16 = pa2T[:].bitcast(F16)
            TR(pa2T16[:, 0:128], a2[:, s, :], ct['identh'][:])
            a2T = T([128, 128], "sp2_a2T")
            S.copy(a2T[:], pa2T16[:, 0:128])
            o2 = T([128, 2, 128], "sp2_o2")
            for ot in range(2):
                po2 = P256()
                MM(po2[:, 0:128], v2[:, ot * 128:(ot + 1) * 128], a2T[:],
                                 start=True, stop=True)
                S.copy(o2[:, ot, :], po2[:, 0:128])
            po3 = P256()
            for ot in range(2):
                MM(po3[:], o2[:, ot, :], wt['soT'][:, ot, :],
                                 start=(ot == 0), stop=(ot == 1))
            t3 = T([128, 256], "sp2_t3")
            V.tensor_add(t3[:], po3[:], wt['sobB'][:])
            V.tensor_add(xs2[:, s, :], t3[:], xa[:, s, :])
        tap("xs2", lambda: tap_batched(xs2, [128, L]))

        # ================= downsample =================
        pds = psD.tile([64, 256], F32, tag="ds", name="ds")
        invr = T([1, BPC, L], "irow_raw", I32)
        dma(invr[:], inv[None, :, :])
        invf = T([1, BPC, L], "irow_f")
        V.tensor_copy(invf[:], invr[:])
        for s in range(BPC):
            # inverse permutation (argsort-based) one-hot
            invB = P512()
            MM(invB[:, 0:L], onesrow1, invf[:, s, :], start=True, stop=True)
            QT = T([128, 2, 256], "perm_oh")
            for tt in range(2):
                V.tensor_scalar(QT[:, tt, :], invB[:, 0:L],
                                ct['iotaC'][:, tt:tt + 1], None,
                                op0=ALU.is_equal)
            tmv = T([128, 2, 128], "tm_tmp")
            for tt in range(2):
                ptr = P256()
                ptr16 = ptr[:].bitcast(F16)
                TR(ptr16[:, 0:128], xs2[:, s, tt * 128:(tt + 1) * 128],
                   ct['identh'][:])
                S.copy(tmv[:, tt, :], ptr16[:, 0:128])
            pxr = P256()
            for tt in range(2):
                MM(pxr[:], tmv[:, tt, :], QT[:, tt, :],
                                 start=(tt == 0), stop=(tt == 1))
            xrp = T([128, 324], "ds_xrp")
            V.memset(xrp[:], 0.0)
            xr3 = xrp[:].rearrange("p (h w) -> p h w", h=18)
            S.copy(xr3[:, 1:17, 1:17], pxr[:].rearrange("p (h w) -> p h w", h=16))
            for kh in range(3):
                for kw in range(3):
                    k = kh * 3 + kw
                    cmp_ = T([128, 64], "ds_cmp")
                    V.tensor_copy(cmp_[:].rearrange("p (a b) -> p a b", a=8),
                                  xr3[:, kh:kh + 16:2, kw:kw + 16:2])
                    MM(pds[:, s * 128:(s + 1) * 128],
                                     cmp_[:],
                                     wt['dsw_pk'][:, k, :],
                                     start=(k == 0), stop=(k == 8),
                                     skip_group_check=True)
        view2 = pds[:].rearrange("p (s c) -> p s c", s=2)
        mus = T([64, 2], "ds_mus")
        V.tensor_reduce(mus[:], view2, axis=AX.X, op=ALU.add)
        mean = T([64, 2], "ds_mean")
        V.tensor_scalar(mean[:], mus[:], 1.0 / 128, None, op0=ALU.mult)
        sq = T([64, 2, 128], "sq_tmp")
        S.activation(sq[:].rearrange("p s c -> p (s c)"), pds[:], AF.Square)
        ss = T([64, 2], "ds_ss")
        V.tensor_reduce(ss[:], sq[:], axis=AX.X, op=ALU.add)
        m2 = T([64, 2], "ds_m2")
        V.tensor_mul(m2[:], mean[:], mean[:])
        var = T([64, 2], "ds_var")
        V.scalar_tensor_tensor(var[:], ss[:], 1.0 / 128, m2[:],
                               op0=ALU.mult, op1=ALU.subtract)
        lv = T([64, 2], "ds_lv")
        S.activation(lv[:], var[:], AF.Ln, bias=epscol[0:64, 0:1])
        rstd = T([64, 2], "ds_rstd")
        S.activation(rstd[:], lv[:], AF.Exp, scale=-0.5)
        xn = T([64, 2, 128], "ds_xn")
        V.tensor_tensor(xn[:], view2,
                        mean[:].unsqueeze(2).to_broadcast((64, 2, 128)),
                        op=ALU.subtract)
        xr2 = T([64, 2, 128], "ds_t1")
        V.tensor_tensor(xr2[:], xn[:],
                        rstd[:].unsqueeze(2).to_broadcast((64, 2, 128)),
                        op=ALU.mult)
        o1 = T([64, 2, 128], "ds_o1")
        V.tensor_tensor(o1[:], xr2[:],
                        wt['ds_ln_wB'][:].unsqueeze(1).to_broadcast((64, 2, 128)),
                        op=ALU.mult)
        o2 = T([64, 2, 128], "ds_xn")
        V.tensor_tensor(o2[:], o1[:],
                        wt['ds_ln_bB'][:].unsqueeze(1).to_broadcast((64, 2, 128)),
                        op=ALU.add)
        for s in range(BPC):
            dma(out[s].rearrange("h w c -> (h w) c"), o2[:, s, :])

        stk.close()
    return nc, tap_t


# ---------------------------------------------------------------------------
_CACHE = {}


def _get_program(taps=()):
    key = tuple(sorted(taps))
    if key not in _CACHE:
        _CACHE[key] = build_program(taps)
    return _CACHE[key]


def make_inmaps(inputs, taps=()):
    cst = host_constants()
    w = prep_weights(inputs)
    x = np.asarray(inputs['x'], np.float32).reshape(16, C, L)
    idx = np.asarray(inputs['sorted_index'], np.int32)
    inv = np.argsort(idx, axis=1, kind='stable').astype(np.int32)
    in_maps = []
    for c in range(NCORES):
        m = {}
        m.update({k: np.ascontiguousarray(v) for k, v in cst.items()})
        m.update({k: np.ascontiguousarray(v) for k, v in w.items()})
        sl = slice(c * BPC, (c + 1) * BPC)
        m['x2'] = np.ascontiguousarray(x[sl])
        m['idx'] = np.ascontiguousarray(idx[sl])
        m['inv'] = np.ascontiguousarray(inv[sl])
        in_maps.append(m)
    return in_maps


def run(inputs, taps=(), trace=False):
    nc, tap_t = _get_program(taps)
    in_maps = make_inmaps(inputs, taps)
    res = run_bass_kernel_spmd(nc, in_maps, list(range(NCORES)), trace=trace)
    outs = np.concatenate([r['out'] for r in res.results], axis=0)
    tapd = {}
    for name in taps:
        tapd[name] = [r.get('t_' + name) for r in res.results]
    return outs, tapd, res


def kernel(**inputs):
    outs, _, _ = run(inputs)
    return outs



# revision 42
# speedup vs baseline: 1.0022x; 1.0022x over previous
"""Trainium2 Bass kernel for nn_Basic_Block_v1 (spatial/spectral Mamba2 block).

Sharding: data-parallel over batch (16 samples) across 8 NeuronCores,
2 samples per core; all parameters replicated. The SSD scans are computed
in closed quadratic form (masked decay matrix x dt-scaled inputs) so all
heavy math runs on the TensorEngine.
"""
import sys
sys.path.insert(0, '/opt/trn_rl_repo')
import json
import os

import numpy as np

import concourse.bass as bass
import concourse.mybir as mybir
from concourse import tile
from concourse.bass_utils import run_bass_kernel_spmd

F32 = mybir.dt.float32
F16 = mybir.dt.float16
I32 = mybir.dt.int32
AF = mybir.ActivationFunctionType
ALU = mybir.AluOpType
AX = mybir.AxisListType

NCORES = 8
BPC = 2          # batch per core
L = 256          # spatial tokens
C = 128          # channels
H1 = 4           # spa heads
DI1 = 256        # spa d_inner
H2 = 8           # spe heads
DI2 = 512        # spe d_inner
L2 = 128         # spe tokens (channels)
DM2 = 256        # spe d_model (seq positions)
NST = 64         # d_state
EPS = 1e-5

# ---------------------------------------------------------------------------
# walrus in this container supports only ONE sync-wait per instruction;
# split extra waits emitted by the Tile scheduler onto preceding NoOps.
_WAIT_LIMIT = 1
_orig_to_json = bass.Bass.to_json_bytes


def _fix_block(b, ctr):
    insts = b.get('instructions')
    if insts:
        out = []
        for ins in insts:
            si = ins.get('sync_info')
            waits = (si or {}).get('on_wait') or []
            if len(waits) > _WAIT_LIMIT:
                while len(waits) > _WAIT_LIMIT:
                    chunk, waits = waits[:_WAIT_LIMIT], waits[_WAIT_LIMIT:]
                    ctr[0] += 1
                    out.append({
                        "debug": ins.get("debug"),
                        "engine": ins["engine"],
                        "ins": [],
                        "name": f"I-wsplit{ctr[0]}",
                        "opcode": "NoOp",
                        "outs": [],
                        "text_hint": "wsplit",
                        "sync_info": {"on_update": [], "on_wait": chunk},
                    })
                si['on_wait'] = waits
            out.append(ins)
        b['instructions'] = out
    for sb in b.get('blocks') or []:
        _fix_block(sb, ctr)


def _patched_to_json(self, *a, **k):
    raw = _orig_to_json(self, *a, **k)
    d = json.loads(raw)
    ctr = [0]
    for f in d.get('functions', []):
        for b in f.get('blocks', []):
            _fix_block(b, ctr)
    if ctr[0] == 0:
        return raw
    return json.dumps(d).encode()


bass.Bass.to_json_bytes = _patched_to_json


# ---------------------------------------------------------------------------
def _sincos_2d(dim, Hg):
    def e1(d, pos):
        omega = 1.0 / (10000.0 ** (np.arange(d // 2, dtype=np.float64) / (d / 2.0)))
        out = pos[:, None] * omega[None, :]
        return np.concatenate([np.sin(out), np.cos(out)], axis=-1)
    gh, gw = np.meshgrid(np.arange(Hg), np.arange(Hg), indexing='ij')
    emb = np.concatenate([e1(dim // 2, gh.reshape(-1)), e1(dim // 2, gw.reshape(-1))], axis=-1)
    return emb.astype(np.float32)


def host_constants():
    d = {}
    d['pe_fm'] = np.ascontiguousarray(_sincos_2d(C, 16).T)              # [128, 256]
    d['ident'] = np.eye(128, dtype=np.float32)
    d['identh'] = np.eye(128, dtype=np.float16)
    iota = np.arange(L, dtype=np.float32)
    d['iotaC'] = np.stack([iota[:128], iota[128:]], axis=1).copy()      # [128, 2]
    # maskT[st][sp][t] = 1 if (st*128+sp) <= t   (spa, L=256)
    sidx = np.arange(L)[:, None]
    tidx = np.arange(L)[None, :]
    m = (sidx <= tidx).astype(np.float32)                               # [s, t]
    d['maskT_spa'] = np.stack([m[:128], m[128:]], axis=1).copy()        # [128, 2, 256]
    s2 = np.arange(L2)[:, None]
    t2 = np.arange(L2)[None, :]
    d['maskT_spe'] = (s2 <= t2).astype(np.float32)                      # [128, 128]
    # head one-hots for dt broadcast: E[k, j, m] = 1 iff k == 2j + m//64
    E1 = np.zeros((H1, 2, 128), np.float32)
    for j in range(2):
        for m in range(128):
            E1[2 * j + m // 64, j, m] = 1.0
    d['E_spaJ'] = E1.astype(np.float16)
    E2 = np.zeros((H2, 4, 128), np.float32)
    for j in range(4):
        for m in range(128):
            E2[2 * j + m // 64, j, m] = 1.0
    d['E_speJ'] = E2.astype(np.float16)
    EA = np.zeros((8, 128), np.float32)
    for h in range(8):
        EA[h, h * 16:(h + 1) * 16] = 1.0
    d['E_attn'] = EA.astype(np.float16)                                 # [8, 128]

    d['Emask_q'] = EA.T.copy()                                          # [128, 8]
    d['onesrow'] = np.ones(512, np.float16)
    return d


COL_ORDER = (
    ["spa_dtb0", "spa_alog0", "spa_cb0_0", "spa_cb0_1", "spa_cbBC0",
     "spa_dpc0_0", "spa_dpc0_1", "spa_rwc0_0", "spa_rwc0_1",
     "spa_dtb1", "spa_alog1", "spa_cb1_0", "spa_cb1_1", "spa_cbBC1",
     "spa_dpc1_0", "spa_dpc1_1", "spa_rwc1_0", "spa_rwc1_1"]
    + ["spe_dtb0", "spe_alog0", "spe_cb0_0", "spe_cb0_1", "spe_cb0_2", "spe_cb0_3",
       "spe_cbBC0",
       "spe_dpc0_0", "spe_dpc0_1", "spe_dpc0_2", "spe_dpc0_3",
       "spe_rwc0_0", "spe_rwc0_1", "spe_rwc0_2", "spe_rwc0_3",
       "spe_dtb1", "spe_alog1", "spe_cb1_0", "spe_cb1_1", "spe_cb1_2", "spe_cb1_3",
       "spe_cbBC1",
       "spe_dpc1_0", "spe_dpc1_1", "spe_dpc1_2", "spe_dpc1_3",
       "spe_rwc1_0", "spe_rwc1_1", "spe_rwc1_2", "spe_rwc1_3"]
    + ["lnw_spa0", "lnw_spa1", "lnw_norm",
       "cprj_b", "aq_b", "ak_b", "av_b", "ao_b",
       "sq_b0", "sq_b1", "sk_b0", "sk_b1"]
)
CIDX = {k: ix for ix, k in enumerate(COL_ORDER)}


F16_WEIGHTS = (
    'spa_in_wT', 'spa_out_pk', 'spe_in_pk', 'spe_out_pk', 'cprj_pk',
    'aqT', 'akT', 'avT', 'aoT', 'sqT', 'skT', 'svT', 'soT', 'dsw_pk', 'lnwb')


def prep_weights(inp):
    """Host-side layout prep of the replicated parameters (tile layouts,
    single DMA per tensor)."""
    w = {}
    w['spa_in_wT'] = np.ascontiguousarray(np.transpose(inp['spa_in_w'], (0, 2, 1)))
    cv = np.zeros((128, 2, 3, 4), np.float32)
    for i in range(2):
        cv[:, i, 0] = inp['spa_conv_w'][i, 0:128]
        cv[:, i, 1] = inp['spa_conv_w'][i, 128:256]
        cv[0:64, i, 2] = inp['spa_conv_w'][i, 256:320]
        cv[64:128, i, 2] = inp['spa_conv_w'][i, 320:384]
    w['spa_conv_pk'] = cv
    sow = np.transpose(inp['spa_out_w'], (0, 2, 1)).reshape(2, 2, 128, 128)
    w['spa_out_pk'] = np.ascontiguousarray(sow.transpose(2, 0, 1, 3))
    w['spe_ln_wB'] = np.ascontiguousarray(np.broadcast_to(
        inp['spe_ln_w'][:, None, :], (2, 128, 256)).transpose(1, 0, 2))
    w['spe_ln_bB'] = np.ascontiguousarray(np.broadcast_to(
        inp['spe_ln_b'][:, None, :], (2, 128, 256)).transpose(1, 0, 2))
    siw = np.transpose(inp['spe_in_w'], (0, 2, 1)).reshape(2, 2, 128, 1160)
    w['spe_in_pk'] = np.ascontiguousarray(siw.transpose(0, 2, 1, 3))
    cv2 = np.zeros((128, 2, 5, 4), np.float32)
    for i in range(2):
        for j in range(4):
            cv2[:, i, j] = inp['spe_conv_w'][i, j * 128:(j + 1) * 128]
        cv2[0:64, i, 4] = inp['spe_conv_w'][i, 512:576]
        cv2[64:128, i, 4] = inp['spe_conv_w'][i, 576:640]
    w['spe_conv_pk'] = cv2
    sew = np.transpose(inp['spe_out_w'], (0, 2, 1)).reshape(2, 4, 128, 256)
    w['spe_out_pk'] = np.ascontiguousarray(sew.transpose(0, 2, 1, 3))
    w['cprj_pk'] = np.ascontiguousarray(
        np.transpose(inp['cprj_w'], (2, 1, 0)).transpose(1, 0, 2))
    for nm in ('aq', 'ak', 'av', 'ao'):
        w[nm + 'T'] = np.ascontiguousarray(inp[nm + '_w'].T)
    for nm in ('sq', 'sk', 'sv', 'so'):
        wt_ = inp[nm + '_w'].T.reshape(2, 128, 256)
        w[nm + 'T'] = np.ascontiguousarray(wt_.transpose(1, 0, 2))
    w['svbB'] = np.ascontiguousarray(np.broadcast_to(inp['sv_b'][None, :], (128, 256)))
    w['sobB'] = np.ascontiguousarray(np.broadcast_to(inp['so_b'][None, :], (128, 256)))
    w['dsw_pk'] = np.ascontiguousarray(
        inp['ds_conv_w'].reshape(9, 128, 128).transpose(1, 0, 2))
    w['ds_ln_wB'] = np.ascontiguousarray(np.broadcast_to(inp['ds_ln_w'][None, :], (64, 128)))
    w['ds_ln_bB'] = np.ascontiguousarray(np.broadcast_to(inp['ds_ln_b'][None, :], (64, 128)))
    lnwb = np.zeros((2, 3, 128), np.float32)
    lnwb[0, 0], lnwb[1, 0] = inp['spa_ln_w'][0], inp['spa_ln_b'][0]
    lnwb[0, 1], lnwb[1, 1] = inp['spa_ln_w'][1], inp['spa_ln_b'][1]
    lnwb[0, 2], lnwb[1, 2] = inp['norm_w'], inp['norm_b']
    w['lnwb'] = lnwb
    cols = {}
    for i in range(2):
        cols[f"spa_dtb{i}"] = inp['spa_dt_bias'][i]
        cols[f"spa_alog{i}"] = np.exp(inp['spa_A_log'][i])
        cols[f"spa_cb{i}_0"] = inp['spa_conv_b'][i, 0:128]
        cols[f"spa_cb{i}_1"] = inp['spa_conv_b'][i, 128:256]
        cols[f"spa_cbBC{i}"] = inp['spa_conv_b'][i, 256:384]
        for j in range(2):
            cols[f"spa_dpc{i}_{j}"] = np.repeat(inp['spa_D'][i], 64)[j * 128:(j + 1) * 128]
            cols[f"spa_rwc{i}_{j}"] = inp['spa_rms_w'][i, j * 128:(j + 1) * 128]
        cols[f"spe_dtb{i}"] = inp['spe_dt_bias'][i]
        cols[f"spe_alog{i}"] = np.exp(inp['spe_A_log'][i])
        for j in range(4):
            cols[f"spe_cb{i}_{j}"] = inp['spe_conv_b'][i, j * 128:(j + 1) * 128]
            cols[f"spe_dpc{i}_{j}"] = np.repeat(inp['spe_D'][i], 64)[j * 128:(j + 1) * 128]
            cols[f"spe_rwc{i}_{j}"] = inp['spe_rms_w'][i, j * 128:(j + 1) * 128]
        cols[f"spe_cbBC{i}"] = inp['spe_conv_b'][i, 512:640]
    cols["lnw_spa0"] = inp['spa_ln_w'][0]
    cols["lnw_spa1"] = inp['spa_ln_w'][1]
    cols["lnw_norm"] = inp['norm_w']
    cols["cprj_b"] = inp['cprj_b']
    for nm in ('aq', 'ak', 'av', 'ao'):
        cols[nm + "_b"] = inp[nm + '_b']
    cols["sq_b0"] = inp['sq_b'][0:128]
    cols["sq_b1"] = inp['sq_b'][128:256]
    cols["sk_b0"] = inp['sk_b'][0:128]
    cols["sk_b1"] = inp['sk_b'][128:256]
    pk = np.zeros((128, len(COL_ORDER)), np.float32)
    for k, v in cols.items():
        v = np.asarray(v, np.float32)
        pk[0:v.shape[0], CIDX[k]] = v
    w['colpak'] = pk
    for k in F16_WEIGHTS:
        w[k] = w[k].astype(np.float16)
    return w



# ---------------------------------------------------------------------------
def build_program(taps=()):
    """Builds the per-core SPMD Bass program. `taps` is a set of intermediate
    names to also write to DRAM outputs (debug only)."""
    nc = bass.Bass()

    def din(name, shape, dt=F32):
        return nc.dram_tensor(name, shape, dt, kind="ExternalInput")

    x2 = din("x2", [BPC, C, L])
    idx = din("idx", [BPC, L], I32)
    inv = din("inv", [BPC, L], I32)

    cst = host_constants()
    cst_t = {k: din(k, list(v.shape), F16 if v.dtype == np.float16 else F32)
             for k, v in cst.items()}

    wnames = {
        'spa_in_wT': [2, 128, 644], 'spa_conv_pk': [128, 2, 3, 4],
        'spa_out_pk': [128, 2, 2, 128],
        'spe_ln_wB': [128, 2, 256], 'spe_ln_bB': [128, 2, 256],
        'spe_in_pk': [2, 128, 2, 1160], 'spe_conv_pk': [128, 2, 5, 4],
        'spe_out_pk': [2, 128, 4, 256],
        'cprj_pk': [128, 5, 128],
        'aqT': [128, 128], 'akT': [128, 128], 'avT': [128, 128], 'aoT': [128, 128],
        'sqT': [128, 2, 256], 'skT': [128, 2, 256], 'svT': [128, 2, 256],
        'soT': [128, 2, 256], 'svbB': [128, 256], 'sobB': [128, 256],
        'dsw_pk': [128, 9, 128], 'ds_ln_wB': [64, 128], 'ds_ln_bB': [64, 128],
        'lnwb': [2, 3, 128], 'colpak': [128, len(COL_ORDER)],
    }
    w_t = {k: din(k, shp, F16 if k in F16_WEIGHTS else F32)
           for k, shp in wnames.items()}

    out = nc.dram_tensor("out", [BPC, 8, 8, C], F32, kind="ExternalOutput")
    tap_t = {}

    with tile.TileContext(nc) as tc:
        import contextlib
        stk = contextlib.ExitStack()
        sb = stk.enter_context(tc.tile_pool(name="sb", bufs=1))
        ps1 = stk.enter_context(tc.tile_pool(name="ps1", bufs=2, space="PSUM"))
        ps2 = stk.enter_context(tc.tile_pool(name="ps2", bufs=3, space="PSUM"))
        psS = stk.enter_context(tc.tile_pool(name="psS", bufs=2, space="PSUM"))
        psD = stk.enter_context(tc.tile_pool(name="psD", bufs=1, space="PSUM"))

        BUFS2 = {"cv_a0", "cv_a1", "rowA", "rowB", "tm_tmp", "ssd_Dt",
                 "perm_oh", "ssd_MT", "spa_xtm",
                 "spe_xtm", "spa_ygt", "spa_ynt", "spe_ygt",
                 "spe_y0", "spe_ynt", "spa_acumT",
                 "spe_acumT", "xc_0", "xc_1", "xc_2", "xc_3", "xc_B", "xc_C",
                 "cv_x2", "cv_x3",
                 "spe_h2sb", "x2f_tmp", "sp2_q2", "sp2_k2", "sp2_v2",
                 "sp2_a2", "sp2_a2T", "sp2_o2", "sp2_ex", "at_ex", "at_aw",
                 "mb_dtv", "mb_acum", "pball",
                 "spe_xn", "spe_u", "spe_xsn", "ds_cmp", "rowC", "ln_rstd",
                 "ln_out"}
        F16TAGS = {
            "ones4", "irow_f", "perm_oh", "tm_tmp", "ln_rhs", "ln_rstd",
            "ln_out", "w_spa_in", "w_spe_in", "w_spe_out", "mb_dtv", "rowC",
            "xc_B", "xc_C", "ssd_MT", "spa_xtm", "spe_xtm", "spa_ynt",
            "spe_ynt", "x2f_tmp", "at_ctr", "at_q", "at_qd", "at_K", "at_vo",
            "at_aw", "sp2_q2", "sp2_k2", "sp2_v2", "sp2_a2T", "sp2_o2",
            "ds_cmp", "ds_xrp", "c_identh", "x0", "mb_xp", "spe_xsn", "xs2", "sp2_a2",
            "spe_h2sb", "cv_x0", "cv_x1", "cv_x2", "cv_x3", "cv_B", "cv_C",
            "cv_a0", "cv_a1", "xc_0", "xc_1", "xc_2", "xc_3", "mb_zsil",
            "spa_ygt", "spe_ygt", "spa_y0t", "spe_y0", "sqy16", "xf16",
            "sq16", "ssd_Et", "ssd_m0m",
        }
        F16TAGS.update("w_" + k for k in F16_WEIGHTS)
        F16TAGS.update("c_" + k for k in ("E_spaJ", "E_speJ", "E_attn"))

        def T(shape, tag, dt=None):
            if dt is None:
                dt = F16 if tag in F16TAGS else F32
            return sb.tile(shape, dt, tag=tag, name=tag,
                           bufs=2 if tag in BUFS2 else 1)

        def P512(tag="b512"):
            return ps1.tile([128, 512], F32, tag=tag, name=tag)

        def P256(tag="b256"):
            return ps2.tile([128, 256], F32, tag=tag, name=tag)

        def tap(name, ap_fn):
            # ap_fn: callable giving (dram_shape, writer) – writer(dram) DMAs data
            if name in taps:
                shape, writer = ap_fn()
                t = nc.dram_tensor("t_" + name, shape, F32, kind="ExternalOutput")
                tap_t[name] = t
                writer(t)

        dma = nc.sync.dma_start
        V = nc.vector
        S = nc.scalar
        G = nc.gpsimd

        def MM(out, lhsT, rhs, **kw):
            return nc.tensor.matmul(out, lhsT, rhs, **kw)

        def TR(out, in_, identity, **kw):
            return nc.tensor.matmul(out, in_, identity, is_transpose=True, **kw)

        # ---------- load constants (stage0-critical first) ----------
        ct = {}

        def load_c(names):
            for k in names:
                if k in ct or k == 'onesrow':
                    continue
                ct[k] = T(list(cst[k].shape), "c_" + k)
                dma(ct[k][:], cst_t[k][:])

        load_c(['pe_fm', 'iotaC', 'ident', 'identh'])
        ones32 = T([128, 128], "ones32")
        V.memset(ones32[:], 1.0)
        onescol32 = ones32[:, 0:1]
        onesrow32 = ones32[0:1, :]

        # ---------- preload weights (staged: mamba weights now, attention
        # and downsample weights deferred until after stage0 issue order) ----
        wt = {}

        def load_w(names):
            for name in names:
                if name in wt or name in ('spa_in_wT', 'spe_in_pk',
                                          'spe_out_pk'):
                    continue
                t = T(wnames[name], "w_" + name)
                dma(t[:], w_t[name][:])
                wt[name] = t

        load_c(list(cst.keys()))
        load_w(['colpak', 'lnwb', 'spa_conv_pk', 'spa_out_pk'])
        colpak = wt['colpak']

        def col(key, p=128):
            return colpak[0:p, CIDX[key]:CIDX[key] + 1]

        ones4 = T([128, 128], "ones4")
        V.memset(ones4[:], 1.0)
        epscol = T([128, 1], "epscol")
        V.memset(epscol[:], EPS)
        onescol = ones4[:, 0:1]       # [128,1]
        onesrow1 = ones4[0:1, :]      # [1,128]
        ident = ct['ident']

        # ---------- stage 0: embed + permute ----------
        xb = T([128, BPC, L], "xb")
        for s in range(BPC):
            dma(xb[:, s, :], x2[s])
        x0 = T([128, BPC, L], "x0")
        V.tensor_tensor(
            x0[:], xb[:],
            ct['pe_fm'][:].unsqueeze(1).to_broadcast((128, BPC, L)),
            op=ALU.add)

        idxr = T([1, BPC, L], "irow_raw", I32)
        dma(idxr[:], idx[None, :, :])
        idxf = T([1, BPC, L], "irow_f")
        V.tensor_copy(idxf[:], idxr[:])

        xs = T([128, BPC, L], "xs")
        for s in range(BPC):
            # PmT[st][sp][t] = (idx[t] == st*128+sp)
            idxB = P512()
            MM(idxB[:, 0:L], onesrow1, idxf[:, s, :], start=True, stop=True)
            PmT = T([128, 2, L], "perm_oh")
            for st in range(2):
                V.tensor_scalar(PmT[:, st, :], idxB[:, 0:L],
                                ct['iotaC'][:, st:st + 1], None,
                                op0=ALU.is_equal)
            # x0 token-major
            x0tm = T([128, 2, 128], "tm_tmp")
            for tt in range(2):
                ptr = P256()
                ptr16 = ptr[:].bitcast(F16)
                TR(ptr16[:, 0:128], x0[:, s, tt * 128:(tt + 1) * 128],
                   ct['identh'][:])
                S.copy(x0tm[:, tt, :], ptr16[:, 0:128])
            pxs = P256()
            for st in range(2):
                MM(pxs[:], x0tm[:, st, :], PmT[:, st, :],
                                 start=(st == 0), stop=(st == 1))
            S.copy(xs[:, s, :], pxs[:])

        def tap_batched(t_sb, shape_per_s):
            def writer(dram):
                for s in range(BPC):
                    dma(dram[s], t_sb[:, s, :])
            return ([BPC] + shape_per_s, writer)

        tap("xs0", lambda: tap_batched(xs, [128, L]))

        load_w(['spe_ln_wB', 'spe_ln_bB', 'spe_conv_pk'])

        # ================= shared helpers =================
        lnrhs = T([2, 512], "ln_rhs")
        dma(lnrhs[1:2, :], cst_t['onesrow'][None, :])

        def part_ln(xflat, lnidx):
            """LayerNorm over the channel (partition) dim of [128, 512]."""
            xf16 = T([128, 512], "xf16")
            S.copy(xf16[:], xflat)
            sq = T([128, 512], "sq16")
            S.activation(sq[:], xf16[:], AF.Square)
            msum = psS.tile([1, 512], F32, tag="small", name="small")
            MM(msum[:], ones4[:, 0:1], xf16[:], start=True, stop=True)
            murow = T([1, 512], "ln_mu")
            V.tensor_scalar(murow[:], msum[:], 1.0 / 128, None, op0=ALU.mult)
            ssum = psS.tile([1, 512], F32, tag="small", name="small")
            MM(ssum[:], ones4[:, 0:1], sq[:], start=True, stop=True)
            mu2 = T([1, 512], "rowA")
            V.tensor_mul(mu2[:], murow[:], murow[:])
            var = T([1, 512], "rowB")
            V.scalar_tensor_tensor(var[:], ssum[:], 1.0 / 128, mu2[:],
                                   op0=ALU.mult, op1=ALU.subtract)
            lnv = T([1, 512], "rowA")
            S.activation(lnv[:], var[:], AF.Ln, bias=epscol[0:1, 0:1])
            rstd = T([1, 512], "ln_rstd")
            S.activation(rstd[:], lnv[:], AF.Exp, scale=-0.5)
            V.scalar_tensor_tensor(lnrhs[0:1, :], murow[:], -1.0, rstd[:],
                                   op0=ALU.mult, op1=ALU.mult)
            Rp = P512()
            MM(Rp[:], wt['lnwb'][:, lnidx, :], lnrhs[:],
                             start=True, stop=True)
            rstdB = P512()
            MM(rstdB[:], onesrow1, rstd[:], start=True, stop=True)
            wcol = col(("lnw_spa0", "lnw_spa1", "lnw_norm")[lnidx])
            tmp = T([128, 512], "ln_tmp")
            V.tensor_mul(tmp[:], xflat, rstdB[:])
            xln = T([128, 512], "ln_out")
            V.scalar_tensor_tensor(xln[:], tmp[:], wcol, Rp[:],
                                   op0=ALU.mult, op1=ALU.add)
            return xln

        def convchain(buf, wc, cb, P, W, tag, E=None):
            """Causal depthwise conv (k=4) + silu. buf [P, 2, W+3]; returns [P, 2, W]."""
            E = E or V
            a0 = T([P, 2, W], "cv_a0")
            E.tensor_scalar(a0[:], buf[:, :, 0:W], wc[:, 0:1], None, op0=ALU.mult)
            a1 = T([P, 2, W], "cv_a1")
            E.scalar_tensor_tensor(a1[:], buf[:, :, 1:W + 1], wc[:, 1:2], a0[:],
                                   op0=ALU.mult, op1=ALU.add)
            a2 = T([P, 2, W], "cv_a0")
            E.scalar_tensor_tensor(a2[:], buf[:, :, 2:W + 2], wc[:, 2:3], a1[:],
                                   op0=ALU.mult, op1=ALU.add)
            a3 = T([P, 2, W], "cv_a1")
            E.scalar_tensor_tensor(a3[:], buf[:, :, 3:W + 3], wc[:, 3:4], a2[:],
                                   op0=ALU.mult, op1=ALU.add)
            xc = T([P, 2, W], tag)
            S.activation(xc[:], a3[:], AF.Silu, bias=cb[:, 0:1])
            return xc

        # ================= spa mamba =================
        def spa_mamba(i, xs):
            xflat = xs[:].rearrange("p s t -> p (s t)")
            xln = part_ln(xflat, i)
            tap(f"xln{i}", lambda: ([128, 512], lambda d: dma(d[:], xln[:])))
            inw_t = T([128, 644], "w_spa_in")
            dma(inw_t[:], w_t['spa_in_wT'][i])
            inw = inw_t[:]
            # dt chain first: keeps scalar engine in the ln/exp table while
            # part_ln's exp is still resident, before the silu block
            pdt = psS.tile([4, 512], F32, tag="small", name="small")
            MM(pdt[:], inw[:, 640:644], xln[:], start=True, stop=True)
            e1 = T([4, 512], "rowA")
            S.activation(e1[:], pdt[:], AF.Exp, bias=col(f"spa_dtb{i}", 4))
            # softplus via ln(1+u) Taylor (|u|<0.5): keeps scalar engine out
            # of the Ln table mid-silu-run
            u2 = T([4, 512], "rowB")
            V.tensor_mul(u2[:], e1[:], e1[:])
            u3 = T([4, 512], "tay3")
            V.tensor_mul(u3[:], u2[:], e1[:])
            u4 = T([4, 512], "tay4")
            V.tensor_mul(u4[:], u2[:], u2[:])
            u5 = T([4, 512], "tay5")
            V.tensor_mul(u5[:], u2[:], u3[:])
            d1 = T([4, 512], "tay6")
            V.scalar_tensor_tensor(d1[:], u2[:], -0.5, e1[:],
                                   op0=ALU.mult, op1=ALU.add)
            d2 = T([4, 512], "rowB")
            V.scalar_tensor_tensor(d2[:], u3[:], 1.0 / 3, d1[:],
                                   op0=ALU.mult, op1=ALU.add)
            d3 = T([4, 512], "tay3")
            V.scalar_tensor_tensor(d3[:], u4[:], -0.25, d2[:],
                                   op0=ALU.mult, op1=ALU.add)
            dtv = T([4, 512], "mb_dtv")
            V.scalar_tensor_tensor(dtv[:], u5[:], 0.2, d3[:],
                                   op0=ALU.mult, op1=ALU.add)
            dtA = T([4, 512], "rowA")
            V.tensor_scalar(dtA[:], dtv[:], col(f"spa_alog{i}", 4), -1.0,
                            op0=ALU.mult, op1=ALU.mult)
            acum = T([4, 512], "mb_acum")
            aflat = T([1, 2, 1024], "aflat")
            for s in range(BPC):
                V.tensor_tensor_scan(acum[:, s * 256:(s + 1) * 256],
                                     dtA[:, s * 256:(s + 1) * 256],
                                     dtA[:, s * 256:(s + 1) * 256], 0.0,
                                     op0=ALU.add, op1=ALU.bypass)
                dma(aflat[0:1, s, :].rearrange("o (p f) -> o p f", p=4),
                    acum[:, s * 256:(s + 1) * 256])
            # in_proj: z (2 blocks), x (2 blocks), B, C
            zsil = T([128, 2, 512], "mb_zsil")
            for j in range(2):
                pz = P512()
                MM(pz[:], inw[:, j * 128:(j + 1) * 128], xln[:],
                                 start=True, stop=True)
                S.activation(zsil[:, j, :], pz[:], AF.Silu)
            cvx = []
            for j in range(2):
                px = P512()
                MM(px[:], inw[:, 256 + j * 128:256 + (j + 1) * 128], xln[:],
                                 start=True, stop=True)
                buf = T([128, 2, 259], f"cv_x{j}")
                V.memset(buf[:, :, 0:3], 0.0)
                S.copy(buf[:, :, 3:259], px[:].rearrange("p (s t) -> p s t", s=2))
                cvx.append(buf)
            pbc = P512()
            MM(pbc[:], inw[:, 512:640], xln[:], start=True, stop=True)
            bufbc = T([128, 2, 259], "cv_B")
            V.memset(bufbc[:, :, 0:3], 0.0)
            S.copy(bufbc[:, :, 3:259], pbc[:].rearrange("p (s t) -> p s t", s=2))
            tap(f"dtv{i}", lambda: ([4, 512], lambda d: dma(d[:], dtv[:])))
            tap(f"acum{i}", lambda: ([4, 512], lambda d: dma(d[:], acum[:])))
            # conv + silu
            xc = []
            for j in range(2):
                xc.append(convchain(cvx[j], wt['spa_conv_pk'][:, i, j, :],
                                    col(f"spa_cb{i}_{j}"), 128, 256, f"xc_{j}"))
            xcBC = convchain(bufbc, wt['spa_conv_pk'][:, i, 2, :],
                             col(f"spa_cbBC{i}"), 128, 256, "xc_B")
            xcB = xcBC[0:64]
            xcC = T([64, 2, 256], "xc_C")
            dma(xcC[:], xcBC[64:128])
            if i == 0:
                tap("xc00", lambda: ([128, 512], lambda d: dma(
                    d[:], xc[0][:].rearrange("p s t -> p (s t)"))))
                tap("xcB0", lambda: ([64, 512], lambda d: dma(
                    d[:], xcB[:].rearrange("p s t -> p (s t)"))))
                tap("xcC0", lambda: ([64, 512], lambda d: dma(
                    d[:], xcC[:].rearrange("p s t -> p (s t)"))))
            # dt-scaled x (feature-major): xp[:, j, :] = xc[j] * dtB_j
            xp = T([128, 2, 512], "mb_xp")
            for j in range(2):
                pdb = P512()
                MM(pdb[:], ct['E_spaJ'][:, j, :], dtv[:], start=True, stop=True)
                V.tensor_mul(xp[:, j, :],
                             xc[j][:].rearrange("p s t -> p (s t)"), pdb[:])
            h1 = T([128, 2, 256], "h1")
            for s in range(BPC):
                # token-major dt-scaled x: xtm [t(128), st, hp(256)]
                xtm = T([128, 2, 256], "spa_xtm")
                for st in range(2):
                    for j in range(2):
                        ptr = P256()
                        ptr16 = ptr[:].bitcast(F16)
                        TR(
                            ptr16[:, 0:128],
                            xp[:, j, s * 256 + st * 128: s * 256 + (st + 1) * 128],
                            ct['identh'][:])
                        S.copy(xtm[:, st, j * 128:(j + 1) * 128], ptr16[:, 0:128])
                # masked M0^T per s-tile
                m0m = T([128, 2, 256], "ssd_m0m")
                for st in range(2):
                    pm0 = P256()
                    MM(pm0[:], xcB[:, s, st * 128:(st + 1) * 128],
                                     xcC[:, s, :], start=True, stop=True)
                    V.tensor_mul(m0m[:, st, :], pm0[:], ct['maskT_spa'][:, st, :])
                # Acum transposes + strided copy
                acumT = T([128, 2, 4], "spa_acumT")
                for tt in range(2):
                    ptr = P256()
                    TR(ptr[:, 0:4],
                                        acum[:, s * 256 + tt * 128: s * 256 + (tt + 1) * 128],
                                        ident[0:4, 0:4])
                    S.copy(acumT[:, tt, :], ptr[:, 0:4])
                pball = P512()
                MM(pball[:], ones32[0:1, :], aflat[:, s, 0:512],
                   start=True, stop=True)
                pbal2 = P512()
                MM(pbal2[:], ones32[0:1, :], aflat[:, s, 512:1024],
                   start=True, stop=True)
                # Y accumulation per head over s-tiles
                ypsl = [P256(), P256()]
                for st in range(2):
                    Dt = T([128, 4, 256], "ssd_Dt")
                    for h in range(H1):
                        pbx = pball if h < 2 else pbal2
                        V.tensor_scalar(Dt[:, h, :],
                                        pbx[:, (h % 2) * 256:(h % 2 + 1) * 256],
                                        acumT[:, st, h:h + 1], 0.0,
                                        op0=ALU.subtract, op1=ALU.min)
                    Et = T([128, 4, 256], "ssd_Et")
                    S.activation(Et[:].rearrange("p h t -> p (h t)"),
                                 Dt[:].rearrange("p h t -> p (h t)"), AF.Exp)
                    MT = T([128, 4, 256], "ssd_MT")
                    V.tensor_tensor(MT[:], Et[:],
                                    m0m[:, st, :].unsqueeze(1).to_broadcast((128, 4, 256)),
                                    op=ALU.mult)
                    if i == 0 and s == 0 and st == 0:
                        tap("Dt00", lambda: ([128, 1024], lambda d: dma(
                            d[:], Dt[:].rearrange("p h t -> p (h t)"))))
                        tap("MT00", lambda: ([128, 1024], lambda d: dma(
                            d[:], MT[:].rearrange("p h t -> p (h t)"))))
                    for h in range(H1):
                        MM(ypsl[h // 2][(h % 2) * 64:(h % 2) * 64 + 64, :],
                                         xtm[:, st, h * 64:(h + 1) * 64],
                                         MT[:, h, :],
                                         start=(st == 0), stop=(st == 1),
                                         tile_position=(0, (h % 2) * 64),
                                         skip_group_check=True)
                if i == 0 and s == 0:
                    tap("xtm0", lambda: ([128, 512], lambda d: dma(
                        d[:], xtm[:].rearrange("p s t -> p (s t)"))))
                    tap("m0m0", lambda: ([128, 512], lambda d: dma(
                        d[:], m0m[:].rearrange("p s t -> p (s t)"))))
                    tap("acumT0", lambda: ([128, 8], lambda d: dma(
                        d[:], acumT[:].rearrange("p s t -> p (s t)"))))
                    tap("acs0", lambda: ([128, 256], lambda d: dma(d[:], acs[:])))
                ygt = T([128, 2, 256], "spa_ygt")
                y0t = T([128, 2, 256], "spa_y0t")
                for j in range(2):
                    V.scalar_tensor_tensor(y0t[:, j, :], xc[j][:, s, :],
                                           col(f"spa_dpc{i}_{j}"),
                                           ypsl[j][:], op0=ALU.mult, op1=ALU.add)
                    V.tensor_mul(ygt[:, j, :], y0t[:, j, :],
                                 zsil[:, j, s * 256:(s + 1) * 256])
                if i == 0 and s == 0:
                    tap("y00", lambda: ([128, 512], lambda d: dma(
                        d[:], y0t[:].rearrange("p j t -> p (j t)"))))
                    tap("zsil0", lambda: ([128, 1024], lambda d: dma(
                        d[:], zsil[:].rearrange("p j t -> p (j t)"))))
                # gated RMS norm over d_inner
                sqy = T([128, 2, 256], "sqy16")
                S.activation(sqy[:].rearrange("p j t -> p (j t)"),
                             ygt[:].rearrange("p j t -> p (j t)"), AF.Square)
                ssy = psS.tile([1, 256], F32, tag="small", name="small")
                for j in range(2):
                    MM(ssy[:], ones4[:, 0:1], sqy[:, j, :],
                                     start=(j == 0), stop=(j == 1))
                rl = T([1, 256], "rowB")
                S.activation(rl[:], ssy[:], AF.Ln, bias=epscol[0:1, 0:1],
                             scale=1.0 / 256)
                rrow = T([1, 256], "rowC")
                S.activation(rrow[:], rl[:], AF.Exp, scale=-0.5)
                rB = P256()
                MM(rB[:], onesrow1, rrow[:], start=True, stop=True)
                ynt = T([128, 2, 256], "spa_ynt")
                for j in range(2):
                    V.scalar_tensor_tensor(ynt[:, j, :], ygt[:, j, :],
                                           col(f"spa_rwc{i}_{j}"),
                                           rB[:], op0=ALU.mult, op1=ALU.mult)
                if i == 0 and s == 0:
                    tap("ygt0", lambda: ([128, 512], lambda d: dma(
                        d[:], ygt[:].rearrange("p s t -> p (s t)"))))
                    tap("ynt0", lambda: ([128, 512], lambda d: dma(
                        d[:], ynt[:].rearrange("p s t -> p (s t)"))))
                pop = P256()
                for j in range(2):
                    MM(pop[:], wt['spa_out_pk'][:, i, j, :], ynt[:, j, :],
                                     start=(j == 0), stop=(j == 1))
                V.tensor_add(h1[:, s, :], pop[:], xs[:, s, :])
            return h1

        # ================= spe mamba =================
        def spe_mamba(i, h1):
            # LayerNorm over the 256 features (free dim), batched samples
            mus = T([128, 2], "spe_mus")
            V.tensor_reduce(mus[:], h1[:], axis=AX.X, op=ALU.add)
            sq2 = T([128, 512], "sq_tmp")
            S.activation(sq2[:], h1[:].rearrange("p s t -> p (s t)"), AF.Square)
            ss2 = T([128, 2], "spe_ss2")
            V.tensor_reduce(ss2[:], sq2[:].rearrange("p (s t) -> p s t", s=2),
                            axis=AX.X, op=ALU.add)
            mean = T([128, 2], "spe_mean")
            V.tensor_scalar(mean[:], mus[:], 1.0 / 256, None, op0=ALU.mult)
            m2 = T([128, 2], "spe_m2")
            V.tensor_mul(m2[:], mean[:], mean[:])
            var2 = T([128, 2], "spe_var")
            V.scalar_tensor_tensor(var2[:], ss2[:], 1.0 / 256, m2[:],
                                   op0=ALU.mult, op1=ALU.subtract)
            l2t = T([128, 2], "spe_l2")
            S.activation(l2t[:], var2[:], AF.Ln, bias=epscol[:, 0:1])
            rstd2 = T([128, 2], "spe_rstd")
            S.activation(rstd2[:], l2t[:], AF.Exp, scale=-0.5)
            X2f = T([128, 2, 2, 128], "x2f_tmp")
            for s in range(BPC):
                xn = T([128, 256], "spe_xn")
                V.tensor_scalar(xn[:], h1[:, s, :], mean[:, s:s + 1], rstd2[:, s:s + 1],
                                op0=ALU.subtract, op1=ALU.mult)
                u = T([128, 256], "spe_u")
                V.tensor_mul(u[:], xn[:], wt['spe_ln_wB'][:, i, :])
                xsn = T([128, 256], "spe_xsn")
                V.tensor_add(xsn[:], u[:], wt['spe_ln_bB'][:, i, :])
                for ft in range(2):
                    ptr = P256()
                    ptr16 = ptr[:].bitcast(F16)
                    TR(ptr16[:, 0:128], xsn[:, ft * 128:(ft + 1) * 128],
                       ct['identh'][:])
                    S.copy(X2f[:, s, ft, :], ptr16[:, 0:128])
            # in_proj (samples batched along free): out cols ordered (s, t2)
            inw2t = T([128, 2, 1160], "w_spe_in")
            dma(inw2t[:], w_t['spe_in_pk'][i])
            inw2 = inw2t[:]
            ow2t = T([128, 4, 256], "w_spe_out")
            dma(ow2t[:], w_t['spe_out_pk'][i])
            ow2 = ow2t[:]

            def mm2(out_ap, off, width):
                for k in range(2):
                    MM(out_ap,
                                     inw2[:, k, off:off + width],
                                     X2f[:, :, k, :],
                                     start=(k == 0), stop=(k == 1))
            pdt = psS.tile([8, 256], F32, tag="small", name="small")
            for k in range(2):
                MM(pdt[:], inw2[:, k, 1152:1160],
                                 X2f[:, :, k, :], start=(k == 0), stop=(k == 1))
            e1 = T([8, 256], "rowA")
            S.activation(e1[:], pdt[:], AF.Exp, bias=col(f"spe_dtb{i}", 8))
            u2 = T([8, 256], "rowB")
            V.tensor_mul(u2[:], e1[:], e1[:])
            u3 = T([8, 256], "tay3")
            V.tensor_mul(u3[:], u2[:], e1[:])
            u4 = T([8, 256], "tay4")
            V.tensor_mul(u4[:], u2[:], u2[:])
            u5 = T([8, 256], "tay5")
            V.tensor_mul(u5[:], u2[:], u3[:])
            d1 = T([8, 256], "tay6")
            V.scalar_tensor_tensor(d1[:], u2[:], -0.5, e1[:],
                                   op0=ALU.mult, op1=ALU.add)
            d2 = T([8, 256], "rowB")
            V.scalar_tensor_tensor(d2[:], u3[:], 1.0 / 3, d1[:],
                                   op0=ALU.mult, op1=ALU.add)
            d3 = T([8, 256], "tay3")
            V.scalar_tensor_tensor(d3[:], u4[:], -0.25, d2[:],
                                   op0=ALU.mult, op1=ALU.add)
            dtv = T([8, 256], "mb_dtv")
            V.scalar_tensor_tensor(dtv[:], u5[:], 0.2, d3[:],
                                   op0=ALU.mult, op1=ALU.add)
            dtA = T([8, 256], "rowA")
            V.tensor_scalar(dtA[:], dtv[:], col(f"spe_alog{i}", 8), -1.0,
                            op0=ALU.mult, op1=ALU.mult)
            acum = T([8, 256], "mb_acum")
            aflat = T([1, 2, 1024], "aflat")
            for s in range(BPC):
                V.tensor_tensor_scan(acum[:, s * 128:(s + 1) * 128],
                                     dtA[:, s * 128:(s + 1) * 128],
                                     dtA[:, s * 128:(s + 1) * 128], 0.0,
                                     op0=ALU.add, op1=ALU.bypass)
                dma(aflat[0:1, s, :].rearrange("o (p f) -> o p f", p=8),
                    acum[:, s * 128:(s + 1) * 128])
            z2sil = T([128, 4, 256], "mb_zsil")
            for j in range(4):
                pz = P256()
                mm2(pz[:], j * 128, 128)
                S.activation(z2sil[:, j, :], pz[:], AF.Silu)
            cvx2 = []
            for j in range(4):
                px = P256()
                mm2(px[:], 512 + j * 128, 128)
                buf = T([128, 2, 131], f"cv_x{j}")
                V.memset(buf[:, :, 0:3], 0.0)
                S.copy(buf[:, :, 3:131], px[:].rearrange("p (s t) -> p s t", s=2))
                cvx2.append(buf)
            pbc = P256()
            for k in range(2):
                MM(pbc[:], inw2[:, k, 1024:1152],
                   X2f[:, :, k, :], start=(k == 0), stop=(k == 1))
            bufbc = T([128, 2, 131], "cv_B")
            V.memset(bufbc[:, :, 0:3], 0.0)
            S.copy(bufbc[:, :, 3:131], pbc[:].rearrange("p (s t) -> p s t", s=2))
            # conv + silu
            xc2 = []
            for j in range(4):
                xc2.append(convchain(cvx2[j], wt['spe_conv_pk'][:, i, j, :],
                                     col(f"spe_cb{i}_{j}"), 128, 128, f"xc_{j}"))
            xcBC = convchain(bufbc, wt['spe_conv_pk'][:, i, 4, :],
                             col(f"spe_cbBC{i}"), 128, 128, "xc_B")
            xcB = xcBC[0:64]
            xcC = T([64, 2, 128], "xc_C")
            dma(xcC[:], xcBC[64:128])
            # dt-scaled x
            xp2 = T([128, 4, 256], "mb_xp")
            for j in range(4):
                pdb = P256()
                MM(pdb[:], ct['E_speJ'][:, j, :], dtv[:], start=True, stop=True)
                V.tensor_mul(xp2[:, j, :],
                             xc2[j][:].rearrange("p s t -> p (s t)"), pdb[:])
            xs_new = T([128, 2, 256], "xs")
            for s in range(BPC):
                xtm2 = T([128, 512], "spe_xtm")
                for j in range(4):
                    ptr = P256()
                    ptr16 = ptr[:].bitcast(F16)
                    TR(ptr16[:, 0:128],
                       xp2[:, j, s * 128:(s + 1) * 128], ct['identh'][:])
                    S.copy(xtm2[:, j * 128:(j + 1) * 128], ptr16[:, 0:128])
                m0m2 = T([128, 128], "ssd_m0m")
                pm0 = P256()
                MM(pm0[:, 0:128], xcB[:, s, :], xcC[:, s, :],
                                 start=True, stop=True)
                V.tensor_mul(m0m2[:], pm0[:, 0:128], ct['maskT_spe'][:])
                acumT = T([128, 8], "spe_acumT")
                ptr = P256()
                TR(ptr[:, 0:8], acum[:, s * 128:(s + 1) * 128],
                                    ident[0:8, 0:8])
                S.copy(acumT[:], ptr[:, 0:8])
                pball = P512()
                MM(pball[:], ones32[0:1, :], aflat[:, s, 0:512],
                   start=True, stop=True)
                pbal2 = P512()
                MM(pbal2[:], ones32[0:1, :], aflat[:, s, 512:1024],
                   start=True, stop=True)
                Dt = T([128, 8, 128], "ssd_Dt")
                for h in range(H2):
                    pbx = pball if h < 4 else pbal2
                    V.tensor_scalar(Dt[:, h, :],
                                    pbx[:, (h % 4) * 128:(h % 4 + 1) * 128],
                                    acumT[:, h:h + 1], 0.0,
                                    op0=ALU.subtract, op1=ALU.min)
                Et = T([128, 8, 128], "ssd_Et")
                S.activation(Et[:].rearrange("p h t -> p (h t)"),
                             Dt[:].rearrange("p h t -> p (h t)"), AF.Exp)
                MT = T([128, 8, 128], "ssd_MT")
                V.tensor_tensor(MT[:], Et[:],
                                m0m2[:].unsqueeze(1).to_broadcast((128, 8, 128)),
                                op=ALU.mult)
                ygt2 = T([128, 4, 128], "spe_ygt")
                for j in range(4):
                    yp = P256()
                    for hh in range(2):
                        h = 2 * j + hh
                        MM(yp[hh * 64:hh * 64 + 64, 0:128],
                                         xtm2[:, h * 64:(h + 1) * 64],
                                         MT[:, h, :], start=True, stop=True,
                                         tile_position=(0, hh * 64),
                                         skip_group_check=True)
                    y0 = T([128, 128], "spe_y0")
                    V.scalar_tensor_tensor(y0[:], xc2[j][:, s, :],
                                           col(f"spe_dpc{i}_{j}"),
                                           yp[:, 0:128], op0=ALU.mult, op1=ALU.add)
                    V.tensor_mul(ygt2[:, j, :], y0[:],
                                 z2sil[:, j, s * 128:(s + 1) * 128])
                sqy = T([128, 4, 128], "sqy16")
                S.activation(sqy[:].rearrange("p j t -> p (j t)"),
                             ygt2[:].rearrange("p j t -> p (j t)"), AF.Square)
                ssy = psS.tile([1, 128], F32, tag="small", name="small")
                for j in range(4):
                    MM(ssy[:], ones4[:, 0:1], sqy[:, j, :],
                                     start=(j == 0), stop=(j == 3))
                rl = T([1, 128], "rowB")
                S.activation(rl[:], ssy[:], AF.Ln, bias=epscol[0:1, 0:1],
                             scale=1.0 / 512)
                rrow = T([1, 128], "rowC")
                S.activation(rrow[:], rl[:], AF.Exp, scale=-0.5)
                rB = P256()
                MM(rB[:, 0:128], onesrow1, rrow[:], start=True, stop=True)
                ynt = T([128, 4, 128], "spe_ynt")
                for j in range(4):
                    V.scalar_tensor_tensor(ynt[:, j, :], ygt2[:, j, :],
                                           col(f"spe_rwc{i}_{j}"),
                                           rB[:, 0:128], op0=ALU.mult, op1=ALU.mult)
                for ft in range(2):
                    ph2 = P256()
                    for k in range(4):
                        MM(ph2[:, 0:128],
                                         ow2[:, k, ft * 128:(ft + 1) * 128],
                                         ynt[:, k, :], start=(k == 0), stop=(k == 3))
                    h2sb = T([128, 128], "spe_h2sb")
                    S.copy(h2sb[:], ph2[:, 0:128])
                    ptr = P256()
                    ptr16 = ptr[:].bitcast(F16)
                    TR(ptr16[:, 0:128], h2sb[:], ct['identh'][:])
                    V.tensor_add(xs_new[:, s, ft * 128:(ft + 1) * 128],
                                 ptr16[:, 0:128],
                                 h1[:, s, ft * 128:(ft + 1) * 128])
            return xs_new

        # ================= layers =================
        cur = xs
        for i in range(2):
            h1 = spa_mamba(i, cur)
            tap(f"h1_{i}", lambda: tap_batched(h1, [128, L]))
            cur = spe_mamba(i, h1)
            tap(f"xsl{i + 1}", lambda: tap_batched(cur, [128, L]))

        load_w(['cprj_pk', 'aqT', 'akT', 'avT', 'aoT', 'sqT', 'skT', 'svT',
                'soT', 'svbB', 'sobB', 'dsw_pk', 'ds_ln_wB', 'ds_ln_bB'])

        # ================= final LN =================
        xfl = part_ln(cur[:].rearrange("p s t -> p (s t)"), 2)
        xf = xfl[:].rearrange("p (s t) -> p s t", s=BPC)
        tap("xf", lambda: ([BPC, 128, L],
                           lambda d: [dma(d[s], xf[:, s, :]) for s in range(BPC)]))

        # ================= spa attention (center query) =================
        pctr = psS.tile([128, 2], F32, tag="small", name="small")
        for l in range(5):
            MM(pctr[:], wt['cprj_pk'][:, l, :], xf[:, :, l],
                             start=(l == 0), stop=(l == 4))
        ctr = T([128, 2], "at_ctr")
        S.activation(ctr[:], pctr[:], AF.Identity, bias=col("cprj_b"))
        pq = psS.tile([128, 2], F32, tag="small", name="small")
        MM(pq[:], wt['aqT'][:], ctr[:], start=True, stop=True)
        qsb = T([128, 2], "at_q")
        S.activation(qsb[:], pq[:], AF.Identity, bias=col("aq_b"))
        pk = P512()
        MM(pk[:], wt['akT'][:], xfl[:], start=True, stop=True)
        Ksb = T([128, 2, 256], "at_K")
        S.activation(Ksb[:].rearrange("p s t -> p (s t)"), pk[:], AF.Identity,
                     bias=col("ak_b"))
        pv = P512()
        MM(pv[:], wt['avT'][:], xfl[:], start=True, stop=True)
        Vsb = T([128, 2, 256], "at_V")
        S.activation(Vsb[:].rearrange("p s t -> p (s t)"), pv[:], AF.Identity,
                     bias=col("av_b"))
        vo = T([128, 2, 256], "at_vo")
        for s in range(BPC):
            qd = T([128, 8], "at_qd")
            V.tensor_tensor(qd[:], qsb[:, s:s + 1].to_broadcast((128, 8)),
                            ct['Emask_q'][:], op=ALU.mult)
            plg = psS.tile([8, 256], F32, tag="small", name="small")
            MM(plg[:], qd[:], Ksb[:, s, :], start=True, stop=True)
            nm = T([8, 1], "at_nm")
            V.tensor_reduce(nm[:], plg[:], axis=AX.X, op=ALU.max, negate=True)
            nm4 = T([8, 1], "at_nm4")
            V.tensor_scalar(nm4[:], nm[:], 0.25, None, op0=ALU.mult)
            ex = T([8, 256], "at_ex")
            S.activation(ex[:], plg[:], AF.Exp, bias=nm4[:, 0:1], scale=0.25)
            sm = T([8, 1], "at_sm")
            V.tensor_reduce(sm[:], ex[:], axis=AX.X, op=ALU.add)
            rc = T([8, 1], "at_rc")
            V.reciprocal(rc[:], sm[:])
            aw = T([8, 256], "at_aw")
            V.tensor_scalar(aw[:], ex[:], rc[:, 0:1], None, op0=ALU.mult)
            patB = P256()
            MM(patB[:], ct['E_attn'][:], aw[:], start=True, stop=True)
            V.tensor_mul(vo[:, s, :], Vsb[:, s, :], patB[:])
        pao = P512()
        MM(pao[:], wt['aoT'][:], vo[:].rearrange("p s t -> p (s t)"),
                         start=True, stop=True)
        xa = T([128, 2, 256], "xa")
        V.scalar_tensor_tensor(xa[:].rearrange("p s t -> p (s t)"), pao[:],
                               col("ao_b"), xfl[:], op0=ALU.add, op1=ALU.add)
        tap("xa", lambda: tap_batched(xa, [128, L]))

        # ================= spe attention =================
        X2a = T([128, 2, 2, 128], "x2f_tmp")
        for s in range(BPC):
            for ft in range(2):
                ptr = P256()
                TR(ptr[:, 0:128], xa[:, s, ft * 128:(ft + 1) * 128],
                                    ident[:])
                S.copy(X2a[:, s, ft, :], ptr[:, 0:128])
        q2 = T([128, 2, 2, 128], "sp2_q2")
        k2 = T([128, 2, 2, 128], "sp2_k2")
        for ot in range(2):
            pq2 = P256()
            for ft in range(2):
                MM(pq2[:], wt['sqT'][:, ft, ot * 128:(ot + 1) * 128],
                   X2a[:, :, ft, :], start=(ft == 0), stop=(ft == 1))
            for s in range(BPC):
                S.activation(q2[:, s, ot, :], pq2[:, s * 128:(s + 1) * 128],
                             AF.Identity, bias=col(f"sq_b{ot}"))
            pk2 = P256()
            for ft in range(2):
                MM(pk2[:], wt['skT'][:, ft, ot * 128:(ot + 1) * 128],
                   X2a[:, :, ft, :], start=(ft == 0), stop=(ft == 1))
            for s in range(BPC):
                S.activation(k2[:, s, ot, :], pk2[:, s * 128:(s + 1) * 128],
                             AF.Identity, bias=col(f"sk_b{ot}"))
        xs2 = T([128, 2, 256], "xs2")
        for s in range(BPC):
            pv2 = P256()
            for ft in range(2):
                MM(pv2[:], X2a[:, s, ft, :], wt['svT'][:, ft, :],
                                 start=(ft == 0), stop=(ft == 1))
            v2 = T([128, 256], "sp2_v2")
            V.tensor_add(v2[:], pv2[:], wt['svbB'][:])
            pa2 = P256()
            for ot in range(2):
                MM(pa2[:, 0:128], q2[:, s, ot, :], k2[:, s, ot, :],
                                 start=(ot == 0), stop=(ot == 1))
            nm = T([128, 1], "sp2_nm")
            V.tensor_reduce(nm[:], pa2[:, 0:128], axis=AX.X, op=ALU.max, negate=True)
            nm16 = T([128, 1], "sp2_nm16")
            V.tensor_scalar(nm16[:], nm[:], 1.0 / 16, None, op0=ALU.mult)
            ex = T([128, 128], "sp2_ex")
            S.activation(ex[:], pa2[:, 0:128], AF.Exp, bias=nm16[:, 0:1], scale=1.0 / 16)
            sm = T([128, 1], "sp2_sm")
            V.tensor_reduce(sm[:], ex[:], axis=AX.X, op=ALU.add)
            rc = T([128, 1], "sp2_rc")
            V.reciprocal(rc[:], sm[:])
            a2 = T([128, 128], "sp2_a2")
            V.tensor_scalar(a2[:], ex[:], rc[:, 0:1], None, op0=ALU.mult)
            pa2T = P256()
            pa2T16 = pa2T[:].bitcast(F16)
            TR(pa2T16[:, 0:128], a2[:], ct['identh'][:])
            a2T = T([128, 128], "sp2_a2T")
            S.copy(a2T[:], pa2T16[:, 0:128])
            o2 = T([128, 2, 128], "sp2_o2")
            for ot in range(2):
                po2 = P256()
                MM(po2[:, 0:128], v2[:, ot * 128:(ot + 1) * 128], a2T[:],
                                 start=True, stop=True)
                S.copy(o2[:, ot, :], po2[:, 0:128])
            po3 = P256()
            for ot in range(2):
                MM(po3[:], o2[:, ot, :], wt['soT'][:, ot, :],
                                 start=(ot == 0), stop=(ot == 1))
            t3 = T([128, 256], "sp2_t3")
            V.tensor_add(t3[:], po3[:], wt['sobB'][:])
            V.tensor_add(xs2[:, s, :], t3[:], xa[:, s, :])
        tap("xs2", lambda: tap_batched(xs2, [128, L]))

        # ================= downsample =================
        pds = psD.tile([64, 256], F32, tag="ds", name="ds")
        invr = T([1, BPC, L], "irow_raw", I32)
        dma(invr[:], inv[None, :, :])
        invf = T([1, BPC, L], "irow_f")
        V.tensor_copy(invf[:], invr[:])
        for s in range(BPC):
            # inverse permutation (argsort-based) one-hot
            invB = P512()
            MM(invB[:, 0:L], onesrow1, invf[:, s, :], start=True, stop=True)
            QT = T([128, 2, 256], "perm_oh")
            for tt in range(2):
                V.tensor_scalar(QT[:, tt, :], invB[:, 0:L],
                                ct['iotaC'][:, tt:tt + 1], None,
                                op0=ALU.is_equal)
            tmv = T([128, 2, 128], "tm_tmp")
            for tt in range(2):
                ptr = P256()
                ptr16 = ptr[:].bitcast(F16)
                TR(ptr16[:, 0:128], xs2[:, s, tt * 128:(tt + 1) * 128],
                   ct['identh'][:])
                S.copy(tmv[:, tt, :], ptr16[:, 0:128])
            pxr = P256()
            for tt in range(2):
                MM(pxr[:], tmv[:, tt, :], QT[:, tt, :],
                                 start=(tt == 0), stop=(tt == 1))
            xrp = T([128, 324], "ds_xrp")
            V.memset(xrp[:], 0.0)
            xr3 = xrp[:].rearrange("p (h w) -> p h w", h=18)
            S.copy(xr3[:, 1:17, 1:17], pxr[:].rearrange("p (h w) -> p h w", h=16))
            for kh in range(3):
                for kw in range(3):
                    k = kh * 3 + kw
                    cmp_ = T([128, 64], "ds_cmp")
                    V.tensor_copy(cmp_[:].rearrange("p (a b) -> p a b", a=8),
                                  xr3[:, kh:kh + 16:2, kw:kw + 16:2])
                    MM(pds[:, s * 128:(s + 1) * 128],
                                     cmp_[:],
                                     wt['dsw_pk'][:, k, :],
                                     start=(k == 0), stop=(k == 8),
                                     skip_group_check=True)
        view2 = pds[:].rearrange("p (s c) -> p s c", s=2)
        mus = T([64, 2], "ds_mus")
        V.tensor_reduce(mus[:], view2, axis=AX.X, op=ALU.add)
        mean = T([64, 2], "ds_mean")
        V.tensor_scalar(mean[:], mus[:], 1.0 / 128, None, op0=ALU.mult)
        sq = T([64, 2, 128], "sq_tmp")
        S.activation(sq[:].rearrange("p s c -> p (s c)"), pds[:], AF.Square)
        ss = T([64, 2], "ds_ss")
        V.tensor_reduce(ss[:], sq[:], axis=AX.X, op=ALU.add)
        m2 = T([64, 2], "ds_m2")
        V.tensor_mul(m2[:], mean[:], mean[:])
        var = T([64, 2], "ds_var")
        V.scalar_tensor_tensor(var[:], ss[:], 1.0 / 128, m2[:],
                               op0=ALU.mult, op1=ALU.subtract)
        lv = T([64, 2], "ds_lv")
        S.activation(lv[:], var[:], AF.Ln, bias=epscol[0:64, 0:1])
        rstd = T([64, 2], "ds_rstd")
        S.activation(rstd[:], lv[:], AF.Exp, scale=-0.5)
        xn = T([64, 2, 128], "ds_xn")
        V.tensor_tensor(xn[:], view2,
                        mean[:].unsqueeze(2).to_broadcast((64, 2, 128)),
                        op=ALU.subtract)
        xr2 = T([64, 2, 128], "ds_t1")
        V.tensor_tensor(xr2[:], xn[:],
                        rstd[:].unsqueeze(2).to_broadcast((64, 2, 128)),
                        op=ALU.mult)
        o1 = T([64, 2, 128], "ds_o1")
        V.tensor_tensor(o1[:], xr2[:],
                        wt['ds_ln_wB'][:].unsqueeze(1).to_broadcast((64, 2, 128)),
                        op=ALU.mult)
        o2 = T([64, 2, 128], "ds_xn")
        V.tensor_tensor(o2[:], o1[:],
                        wt['ds_ln_bB'][:].unsqueeze(1).to_broadcast((64, 2, 128)),
                        op=ALU.add)
        for s in range(BPC):
            dma(out[s].rearrange("h w c -> (h w) c"), o2[:, s, :])

        stk.close()
    return nc, tap_t


# ---------------------------------------------------------------------------
_CACHE = {}


def _get_program(taps=()):
    key = tuple(sorted(taps))
    if key not in _CACHE:
        _CACHE[key] = build_program(taps)
    return _CACHE[key]


def make_inmaps(inputs, taps=()):
    cst = host_constants()
    w = prep_weights(inputs)
    x = np.asarray(inputs['x'], np.float32).reshape(16, C, L)
    idx = np.asarray(inputs['sorted_index'], np.int32)
    inv = np.argsort(idx, axis=1, kind='stable').astype(np.int32)
    in_maps = []
    for c in range(NCORES):
        m = {}
        m.update({k: np.ascontiguousarray(v) for k, v in cst.items()})
        m.update({k: np.ascontiguousarray(v) for k, v in w.items()})
        sl = slice(c * BPC, (c + 1) * BPC)
        m['x2'] = np.ascontiguousarray(x[sl])
        m['idx'] = np.ascontiguousarray(idx[sl])
        m['inv'] = np.ascontiguousarray(inv[sl])
        in_maps.append(m)
    return in_maps


def run(inputs, taps=(), trace=False):
    nc, tap_t = _get_program(taps)
    in_maps = make_inmaps(inputs, taps)
    res = run_bass_kernel_spmd(nc, in_maps, list(range(NCORES)), trace=trace)
    outs = np.concatenate([r['out'] for r in res.results], axis=0)
    tapd = {}
    for name in taps:
        tapd[name] = [r.get('t_' + name) for r in res.results]
    return outs, tapd, res


def kernel(**inputs):
    outs, _, _ = run(inputs)
    return outs



# revision 44
# speedup vs baseline: 1.0030x; 1.0008x over previous
"""Trainium2 Bass kernel for nn_Basic_Block_v1 (spatial/spectral Mamba2 block).

Sharding: data-parallel over batch (16 samples) across 8 NeuronCores,
2 samples per core; all parameters replicated. The SSD scans are computed
in closed quadratic form (masked decay matrix x dt-scaled inputs) so all
heavy math runs on the TensorEngine.
"""
import sys
sys.path.insert(0, '/opt/trn_rl_repo')
import json
import os

import numpy as np

import concourse.bass as bass
import concourse.mybir as mybir
from concourse import tile
from concourse.bass_utils import run_bass_kernel_spmd

F32 = mybir.dt.float32
F16 = mybir.dt.float16
I32 = mybir.dt.int32
AF = mybir.ActivationFunctionType
ALU = mybir.AluOpType
AX = mybir.AxisListType

NCORES = 8
BPC = 2          # batch per core
L = 256          # spatial tokens
C = 128          # channels
H1 = 4           # spa heads
DI1 = 256        # spa d_inner
H2 = 8           # spe heads
DI2 = 512        # spe d_inner
L2 = 128         # spe tokens (channels)
DM2 = 256        # spe d_model (seq positions)
NST = 64         # d_state
EPS = 1e-5

# ---------------------------------------------------------------------------
# walrus in this container supports only ONE sync-wait per instruction;
# split extra waits emitted by the Tile scheduler onto preceding NoOps.
_WAIT_LIMIT = 1
_orig_to_json = bass.Bass.to_json_bytes


def _fix_block(b, ctr):
    insts = b.get('instructions')
    if insts:
        out = []
        for ins in insts:
            si = ins.get('sync_info')
            waits = (si or {}).get('on_wait') or []
            if len(waits) > _WAIT_LIMIT:
                while len(waits) > _WAIT_LIMIT:
                    chunk, waits = waits[:_WAIT_LIMIT], waits[_WAIT_LIMIT:]
                    ctr[0] += 1
                    out.append({
                        "debug": ins.get("debug"),
                        "engine": ins["engine"],
                        "ins": [],
                        "name": f"I-wsplit{ctr[0]}",
                        "opcode": "NoOp",
                        "outs": [],
                        "text_hint": "wsplit",
                        "sync_info": {"on_update": [], "on_wait": chunk},
                    })
                si['on_wait'] = waits
            out.append(ins)
        b['instructions'] = out
    for sb in b.get('blocks') or []:
        _fix_block(sb, ctr)


def _patched_to_json(self, *a, **k):
    raw = _orig_to_json(self, *a, **k)
    d = json.loads(raw)
    ctr = [0]
    for f in d.get('functions', []):
        for b in f.get('blocks', []):
            _fix_block(b, ctr)
    if ctr[0] == 0:
        return raw
    return json.dumps(d).encode()


bass.Bass.to_json_bytes = _patched_to_json


# ---------------------------------------------------------------------------
def _sincos_2d(dim, Hg):
    def e1(d, pos):
        omega = 1.0 / (10000.0 ** (np.arange(d // 2, dtype=np.float64) / (d / 2.0)))
        out = pos[:, None] * omega[None, :]
        return np.concatenate([np.sin(out), np.cos(out)], axis=-1)
    gh, gw = np.meshgrid(np.arange(Hg), np.arange(Hg), indexing='ij')
    emb = np.concatenate([e1(dim // 2, gh.reshape(-1)), e1(dim // 2, gw.reshape(-1))], axis=-1)
    return emb.astype(np.float32)


def host_constants():
    d = {}
    d['pe_fm'] = np.ascontiguousarray(_sincos_2d(C, 16).T)              # [128, 256]
    d['ident'] = np.eye(128, dtype=np.float32)
    d['identh'] = np.eye(128, dtype=np.float16)
    iota = np.arange(L, dtype=np.float32)
    d['iotaC'] = np.stack([iota[:128], iota[128:]], axis=1).copy()      # [128, 2]
    # maskT[st][sp][t] = 1 if (st*128+sp) <= t   (spa, L=256)
    sidx = np.arange(L)[:, None]
    tidx = np.arange(L)[None, :]
    m = (sidx <= tidx).astype(np.float32)                               # [s, t]
    d['maskT_spa'] = np.stack([m[:128], m[128:]], axis=1).astype(np.float16)  # [128, 2, 256]
    s2 = np.arange(L2)[:, None]
    t2 = np.arange(L2)[None, :]
    d['maskT_spe'] = (s2 <= t2).astype(np.float16)                      # [128, 128]
    # head one-hots for dt broadcast: E[k, j, m] = 1 iff k == 2j + m//64
    E1 = np.zeros((H1, 2, 128), np.float32)
    for j in range(2):
        for m in range(128):
            E1[2 * j + m // 64, j, m] = 1.0
    d['E_spaJ'] = E1.astype(np.float16)
    E2 = np.zeros((H2, 4, 128), np.float32)
    for j in range(4):
        for m in range(128):
            E2[2 * j + m // 64, j, m] = 1.0
    d['E_speJ'] = E2.astype(np.float16)
    EA = np.zeros((8, 128), np.float32)
    for h in range(8):
        EA[h, h * 16:(h + 1) * 16] = 1.0
    d['E_attn'] = EA.astype(np.float16)                                 # [8, 128]

    d['Emask_q'] = EA.T.copy()                                          # [128, 8]
    d['onesrow'] = np.ones(512, np.float16)
    return d


COL_ORDER = (
    ["spa_dtb0", "spa_alog0", "spa_cb0_0", "spa_cb0_1", "spa_cbBC0",
     "spa_dpc0_0", "spa_dpc0_1", "spa_rwc0_0", "spa_rwc0_1",
     "spa_dtb1", "spa_alog1", "spa_cb1_0", "spa_cb1_1", "spa_cbBC1",
     "spa_dpc1_0", "spa_dpc1_1", "spa_rwc1_0", "spa_rwc1_1"]
    + ["spe_dtb0", "spe_alog0", "spe_cb0_0", "spe_cb0_1", "spe_cb0_2", "spe_cb0_3",
       "spe_cbBC0",
       "spe_dpc0_0", "spe_dpc0_1", "spe_dpc0_2", "spe_dpc0_3",
       "spe_rwc0_0", "spe_rwc0_1", "spe_rwc0_2", "spe_rwc0_3",
       "spe_dtb1", "spe_alog1", "spe_cb1_0", "spe_cb1_1", "spe_cb1_2", "spe_cb1_3",
       "spe_cbBC1",
       "spe_dpc1_0", "spe_dpc1_1", "spe_dpc1_2", "spe_dpc1_3",
       "spe_rwc1_0", "spe_rwc1_1", "spe_rwc1_2", "spe_rwc1_3"]
    + ["lnw_spa0", "lnw_spa1", "lnw_norm",
       "cprj_b", "aq_b", "ak_b", "av_b", "ao_b",
       "sq_b0", "sq_b1", "sk_b0", "sk_b1"]
)
CIDX = {k: ix for ix, k in enumerate(COL_ORDER)}


F16_WEIGHTS = (
    'spa_in_wT', 'spa_out_pk', 'spe_in_pk', 'spe_out_pk', 'cprj_pk',
    'aqT', 'akT', 'avT', 'aoT', 'sqT', 'skT', 'svT', 'soT', 'dsw_pk', 'lnwb',
    'spe_ln_wB', 'spe_ln_bB', 'svbB', 'sobB', 'ds_ln_wB', 'ds_ln_bB')


def prep_weights(inp):
    """Host-side layout prep of the replicated parameters (tile layouts,
    single DMA per tensor)."""
    w = {}
    w['spa_in_wT'] = np.ascontiguousarray(np.transpose(inp['spa_in_w'], (0, 2, 1)))
    cv = np.zeros((128, 2, 3, 4), np.float32)
    for i in range(2):
        cv[:, i, 0] = inp['spa_conv_w'][i, 0:128]
        cv[:, i, 1] = inp['spa_conv_w'][i, 128:256]
        cv[0:64, i, 2] = inp['spa_conv_w'][i, 256:320]
        cv[64:128, i, 2] = inp['spa_conv_w'][i, 320:384]
    w['spa_conv_pk'] = cv
    sow = np.transpose(inp['spa_out_w'], (0, 2, 1)).reshape(2, 2, 128, 128)
    w['spa_out_pk'] = np.ascontiguousarray(sow.transpose(2, 0, 1, 3))
    w['spe_ln_wB'] = np.ascontiguousarray(np.broadcast_to(
        inp['spe_ln_w'][:, None, :], (2, 128, 256)).transpose(1, 0, 2))
    w['spe_ln_bB'] = np.ascontiguousarray(np.broadcast_to(
        inp['spe_ln_b'][:, None, :], (2, 128, 256)).transpose(1, 0, 2))
    siw = np.transpose(inp['spe_in_w'], (0, 2, 1)).reshape(2, 2, 128, 1160)
    w['spe_in_pk'] = np.ascontiguousarray(siw.transpose(0, 2, 1, 3))
    cv2 = np.zeros((128, 2, 5, 4), np.float32)
    for i in range(2):
        for j in range(4):
            cv2[:, i, j] = inp['spe_conv_w'][i, j * 128:(j + 1) * 128]
        cv2[0:64, i, 4] = inp['spe_conv_w'][i, 512:576]
        cv2[64:128, i, 4] = inp['spe_conv_w'][i, 576:640]
    w['spe_conv_pk'] = cv2
    sew = np.transpose(inp['spe_out_w'], (0, 2, 1)).reshape(2, 4, 128, 256)
    w['spe_out_pk'] = np.ascontiguousarray(sew.transpose(0, 2, 1, 3))
    w['cprj_pk'] = np.ascontiguousarray(
        np.transpose(inp['cprj_w'], (2, 1, 0)).transpose(1, 0, 2))
    for nm in ('aq', 'ak', 'av', 'ao'):
        w[nm + 'T'] = np.ascontiguousarray(inp[nm + '_w'].T)
    for nm in ('sq', 'sk', 'sv', 'so'):
        wt_ = inp[nm + '_w'].T.reshape(2, 128, 256)
        w[nm + 'T'] = np.ascontiguousarray(wt_.transpose(1, 0, 2))
    w['svbB'] = np.ascontiguousarray(np.broadcast_to(inp['sv_b'][None, :], (128, 256)))
    w['sobB'] = np.ascontiguousarray(np.broadcast_to(inp['so_b'][None, :], (128, 256)))
    w['dsw_pk'] = np.ascontiguousarray(
        inp['ds_conv_w'].reshape(9, 128, 128).transpose(1, 0, 2))
    w['ds_ln_wB'] = np.ascontiguousarray(np.broadcast_to(inp['ds_ln_w'][None, :], (64, 128)))
    w['ds_ln_bB'] = np.ascontiguousarray(np.broadcast_to(inp['ds_ln_b'][None, :], (64, 128)))
    lnwb = np.zeros((2, 3, 128), np.float32)
    lnwb[0, 0], lnwb[1, 0] = inp['spa_ln_w'][0], inp['spa_ln_b'][0]
    lnwb[0, 1], lnwb[1, 1] = inp['spa_ln_w'][1], inp['spa_ln_b'][1]
    lnwb[0, 2], lnwb[1, 2] = inp['norm_w'], inp['norm_b']
    w['lnwb'] = lnwb
    cols = {}
    for i in range(2):
        cols[f"spa_dtb{i}"] = inp['spa_dt_bias'][i]
        cols[f"spa_alog{i}"] = np.exp(inp['spa_A_log'][i])
        cols[f"spa_cb{i}_0"] = inp['spa_conv_b'][i, 0:128]
        cols[f"spa_cb{i}_1"] = inp['spa_conv_b'][i, 128:256]
        cols[f"spa_cbBC{i}"] = inp['spa_conv_b'][i, 256:384]
        for j in range(2):
            cols[f"spa_dpc{i}_{j}"] = np.repeat(inp['spa_D'][i], 64)[j * 128:(j + 1) * 128]
            cols[f"spa_rwc{i}_{j}"] = inp['spa_rms_w'][i, j * 128:(j + 1) * 128]
        cols[f"spe_dtb{i}"] = inp['spe_dt_bias'][i]
        cols[f"spe_alog{i}"] = np.exp(inp['spe_A_log'][i])
        for j in range(4):
            cols[f"spe_cb{i}_{j}"] = inp['spe_conv_b'][i, j * 128:(j + 1) * 128]
            cols[f"spe_dpc{i}_{j}"] = np.repeat(inp['spe_D'][i], 64)[j * 128:(j + 1) * 128]
            cols[f"spe_rwc{i}_{j}"] = inp['spe_rms_w'][i, j * 128:(j + 1) * 128]
        cols[f"spe_cbBC{i}"] = inp['spe_conv_b'][i, 512:640]
    cols["lnw_spa0"] = inp['spa_ln_w'][0]
    cols["lnw_spa1"] = inp['spa_ln_w'][1]
    cols["lnw_norm"] = inp['norm_w']
    cols["cprj_b"] = inp['cprj_b']
    for nm in ('aq', 'ak', 'av', 'ao'):
        cols[nm + "_b"] = inp[nm + '_b']
    cols["sq_b0"] = inp['sq_b'][0:128]
    cols["sq_b1"] = inp['sq_b'][128:256]
    cols["sk_b0"] = inp['sk_b'][0:128]
    cols["sk_b1"] = inp['sk_b'][128:256]
    pk = np.zeros((128, len(COL_ORDER)), np.float32)
    for k, v in cols.items():
        v = np.asarray(v, np.float32)
        pk[0:v.shape[0], CIDX[k]] = v
    w['colpak'] = pk
    for k in F16_WEIGHTS:
        w[k] = w[k].astype(np.float16)
    return w



# ---------------------------------------------------------------------------
def build_program(taps=()):
    """Builds the per-core SPMD Bass program. `taps` is a set of intermediate
    names to also write to DRAM outputs (debug only)."""
    nc = bass.Bass()

    def din(name, shape, dt=F32):
        return nc.dram_tensor(name, shape, dt, kind="ExternalInput")

    x2 = din("x2", [BPC, C, L])
    idx = din("idx", [BPC, L], I32)
    inv = din("inv", [BPC, L], I32)

    cst = host_constants()
    cst_t = {k: din(k, list(v.shape), F16 if v.dtype == np.float16 else F32)
             for k, v in cst.items()}

    wnames = {
        'spa_in_wT': [2, 128, 644], 'spa_conv_pk': [128, 2, 3, 4],
        'spa_out_pk': [128, 2, 2, 128],
        'spe_ln_wB': [128, 2, 256], 'spe_ln_bB': [128, 2, 256],
        'spe_in_pk': [2, 128, 2, 1160], 'spe_conv_pk': [128, 2, 5, 4],
        'spe_out_pk': [2, 128, 4, 256],
        'cprj_pk': [128, 5, 128],
        'aqT': [128, 128], 'akT': [128, 128], 'avT': [128, 128], 'aoT': [128, 128],
        'sqT': [128, 2, 256], 'skT': [128, 2, 256], 'svT': [128, 2, 256],
        'soT': [128, 2, 256], 'svbB': [128, 256], 'sobB': [128, 256],
        'dsw_pk': [128, 9, 128], 'ds_ln_wB': [64, 128], 'ds_ln_bB': [64, 128],
        'lnwb': [2, 3, 128], 'colpak': [128, len(COL_ORDER)],
    }
    w_t = {k: din(k, shp, F16 if k in F16_WEIGHTS else F32)
           for k, shp in wnames.items()}

    out = nc.dram_tensor("out", [BPC, 8, 8, C], F32, kind="ExternalOutput")
    tap_t = {}

    with tile.TileContext(nc) as tc:
        import contextlib
        stk = contextlib.ExitStack()
        sb = stk.enter_context(tc.tile_pool(name="sb", bufs=1))
        ps1 = stk.enter_context(tc.tile_pool(name="ps1", bufs=2, space="PSUM"))
        ps2 = stk.enter_context(tc.tile_pool(name="ps2", bufs=3, space="PSUM"))
        psS = stk.enter_context(tc.tile_pool(name="psS", bufs=2, space="PSUM"))
        psD = stk.enter_context(tc.tile_pool(name="psD", bufs=1, space="PSUM"))

        BUFS2 = {"cv_a0", "cv_a1", "rowA", "rowB", "tm_tmp", "ssd_Dt",
                 "perm_oh", "ssd_MT", "spa_xtm",
                 "spe_xtm", "spa_ygt", "spa_ynt", "spe_ygt",
                 "spe_y0", "spe_ynt", "spa_acumT",
                 "spe_acumT", "xc_0", "xc_1", "xc_2", "xc_3", "xc_B", "xc_C",
                 "cv_x2", "cv_x3",
                 "spe_h2sb", "x2f_tmp", "sp2_q2", "sp2_k2", "sp2_v2",
                 "sp2_a2", "sp2_a2T", "sp2_o2", "sp2_ex", "at_ex", "at_aw",
                 "mb_dtv", "mb_acum", "pball",
                 "spe_xn", "spe_u", "spe_xsn", "ds_cmp", "rowC", "ln_rstd",
                 "ln_out", "w_spa_in", "w_spe_in", "w_spe_out"}
        F16TAGS = {
            "ones4", "irow_f", "perm_oh", "tm_tmp", "ln_rhs", "ln_rstd",
            "ln_out", "w_spa_in", "w_spe_in", "w_spe_out", "mb_dtv", "rowC",
            "xc_B", "xc_C", "ssd_MT", "spa_xtm", "spe_xtm", "spa_ynt",
            "spe_ynt", "x2f_tmp", "at_ctr", "at_q", "at_qd", "at_K", "at_vo",
            "at_aw", "sp2_q2", "sp2_k2", "sp2_v2", "sp2_a2T", "sp2_o2",
            "ds_cmp", "ds_xrp", "c_identh", "x0", "mb_xp", "spe_xsn", "xs2", "sp2_a2",
            "spe_h2sb", "cv_x0", "cv_x1", "cv_x2", "cv_x3", "cv_B", "cv_C",
            "cv_a0", "cv_a1", "xc_0", "xc_1", "xc_2", "xc_3", "mb_zsil",
            "spa_ygt", "spe_ygt", "spa_y0t", "spe_y0", "sqy16", "xf16",
            "sq16", "ssd_Et", "ssd_m0m",
        }
        F16TAGS.update("w_" + k for k in F16_WEIGHTS)
        F16TAGS.update("c_" + k for k in ("E_spaJ", "E_speJ", "E_attn",
                                          "maskT_spa", "maskT_spe"))

        def T(shape, tag, dt=None):
            if dt is None:
                dt = F16 if tag in F16TAGS else F32
            return sb.tile(shape, dt, tag=tag, name=tag,
                           bufs=2 if tag in BUFS2 else 1)

        def P512(tag="b512"):
            return ps1.tile([128, 512], F32, tag=tag, name=tag)

        def P256(tag="b256"):
            return ps2.tile([128, 256], F32, tag=tag, name=tag)

        def tap(name, ap_fn):
            # ap_fn: callable giving (dram_shape, writer) – writer(dram) DMAs data
            if name in taps:
                shape, writer = ap_fn()
                t = nc.dram_tensor("t_" + name, shape, F32, kind="ExternalOutput")
                tap_t[name] = t
                writer(t)

        dma = nc.sync.dma_start
        V = nc.vector
        S = nc.scalar
        G = nc.gpsimd

        def MM(out, lhsT, rhs, **kw):
            return nc.tensor.matmul(out, lhsT, rhs, **kw)

        def TR(out, in_, identity, **kw):
            return nc.tensor.matmul(out, in_, identity, is_transpose=True, **kw)

        # ---------- load constants (stage0-critical first) ----------
        ct = {}

        def load_c(names):
            for k in names:
                if k in ct or k == 'onesrow':
                    continue
                ct[k] = T(list(cst[k].shape), "c_" + k)
                dma(ct[k][:], cst_t[k][:])

        load_c(['pe_fm', 'iotaC', 'ident', 'identh'])
        ones32 = T([128, 128], "ones32")
        V.memset(ones32[:], 1.0)
        onescol32 = ones32[:, 0:1]
        onesrow32 = ones32[0:1, :]

        # ---------- preload weights (staged: mamba weights now, attention
        # and downsample weights deferred until after stage0 issue order) ----
        wt = {}

        def load_w(names):
            for name in names:
                if name in wt or name in ('spa_in_wT', 'spe_in_pk',
                                          'spe_out_pk'):
                    continue
                t = T(wnames[name], "w_" + name)
                dma(t[:], w_t[name][:])
                wt[name] = t

        load_c(list(cst.keys()))
        load_w(['colpak', 'lnwb', 'spa_conv_pk', 'spa_out_pk'])
        colpak = wt['colpak']

        def col(key, p=128):
            return colpak[0:p, CIDX[key]:CIDX[key] + 1]

        ones4 = T([128, 128], "ones4")
        V.memset(ones4[:], 1.0)
        epscol = T([128, 1], "epscol")
        V.memset(epscol[:], EPS)
        onescol = ones4[:, 0:1]       # [128,1]
        onesrow1 = ones4[0:1, :]      # [1,128]
        ident = ct['ident']

        # ---------- stage 0: embed + permute ----------
        xb = T([128, BPC, L], "xb")
        for s in range(BPC):
            dma(xb[:, s, :], x2[s])
        x0 = T([128, BPC, L], "x0")
        V.tensor_tensor(
            x0[:], xb[:],
            ct['pe_fm'][:].unsqueeze(1).to_broadcast((128, BPC, L)),
            op=ALU.add)

        idxr = T([1, BPC, L], "irow_raw", I32)
        dma(idxr[:], idx[None, :, :])
        idxf = T([1, BPC, L], "irow_f")
        V.tensor_copy(idxf[:], idxr[:])

        xs = T([128, BPC, L], "xs")
        for s in range(BPC):
            # PmT[st][sp][t] = (idx[t] == st*128+sp)
            idxB = P512()
            MM(idxB[:, 0:L], onesrow1, idxf[:, s, :], start=True, stop=True)
            PmT = T([128, 2, L], "perm_oh")
            for st in range(2):
                V.tensor_scalar(PmT[:, st, :], idxB[:, 0:L],
                                ct['iotaC'][:, st:st + 1], None,
                                op0=ALU.is_equal)
            # x0 token-major
            x0tm = T([128, 2, 128], "tm_tmp")
            for tt in range(2):
                ptr = P256()
                ptr16 = ptr[:].bitcast(F16)
                TR(ptr16[:, 0:128], x0[:, s, tt * 128:(tt + 1) * 128],
                   ct['identh'][:])
                S.copy(x0tm[:, tt, :], ptr16[:, 0:128])
            pxs = P256()
            for st in range(2):
                MM(pxs[:], x0tm[:, st, :], PmT[:, st, :],
                                 start=(st == 0), stop=(st == 1))
            S.copy(xs[:, s, :], pxs[:])

        def tap_batched(t_sb, shape_per_s):
            def writer(dram):
                for s in range(BPC):
                    dma(dram[s], t_sb[:, s, :])
            return ([BPC] + shape_per_s, writer)

        tap("xs0", lambda: tap_batched(xs, [128, L]))

        load_w(['spe_ln_wB', 'spe_ln_bB', 'spe_conv_pk'])

        # ================= shared helpers =================
        lnrhs = T([2, 512], "ln_rhs")
        dma(lnrhs[1:2, :], cst_t['onesrow'][None, :])

        def part_ln(xflat, lnidx):
            """LayerNorm over the channel (partition) dim of [128, 512]."""
            xf16 = T([128, 512], "xf16")
            S.copy(xf16[:], xflat)
            sq = T([128, 512], "sq16")
            S.activation(sq[:], xf16[:], AF.Square)
            msum = psS.tile([1, 512], F32, tag="small", name="small")
            MM(msum[:], ones4[:, 0:1], xf16[:], start=True, stop=True)
            murow = T([1, 512], "ln_mu")
            V.tensor_scalar(murow[:], msum[:], 1.0 / 128, None, op0=ALU.mult)
            ssum = psS.tile([1, 512], F32, tag="small", name="small")
            MM(ssum[:], ones4[:, 0:1], sq[:], start=True, stop=True)
            mu2 = T([1, 512], "rowA")
            V.tensor_mul(mu2[:], murow[:], murow[:])
            var = T([1, 512], "rowB")
            V.scalar_tensor_tensor(var[:], ssum[:], 1.0 / 128, mu2[:],
                                   op0=ALU.mult, op1=ALU.subtract)
            lnv = T([1, 512], "rowA")
            S.activation(lnv[:], var[:], AF.Ln, bias=epscol[0:1, 0:1])
            rstd = T([1, 512], "ln_rstd")
            S.activation(rstd[:], lnv[:], AF.Exp, scale=-0.5)
            V.scalar_tensor_tensor(lnrhs[0:1, :], murow[:], -1.0, rstd[:],
                                   op0=ALU.mult, op1=ALU.mult)
            Rp = P512()
            MM(Rp[:], wt['lnwb'][:, lnidx, :], lnrhs[:],
                             start=True, stop=True)
            rstdB = P512()
            MM(rstdB[:], onesrow1, rstd[:], start=True, stop=True)
            wcol = col(("lnw_spa0", "lnw_spa1", "lnw_norm")[lnidx])
            tmp = T([128, 512], "ln_tmp")
            V.tensor_mul(tmp[:], xflat, rstdB[:])
            xln = T([128, 512], "ln_out")
            V.scalar_tensor_tensor(xln[:], tmp[:], wcol, Rp[:],
                                   op0=ALU.mult, op1=ALU.add)
            return xln

        def convchain(buf, wc, cb, P, W, tag, E=None):
            """Causal depthwise conv (k=4) + silu. buf [P, 2, W+3]; returns [P, 2, W]."""
            E = E or V
            a0 = T([P, 2, W], "cv_a0")
            E.tensor_scalar(a0[:], buf[:, :, 0:W], wc[:, 0:1], None, op0=ALU.mult)
            a1 = T([P, 2, W], "cv_a1")
            E.scalar_tensor_tensor(a1[:], buf[:, :, 1:W + 1], wc[:, 1:2], a0[:],
                                   op0=ALU.mult, op1=ALU.add)
            a2 = T([P, 2, W], "cv_a0")
            E.scalar_tensor_tensor(a2[:], buf[:, :, 2:W + 2], wc[:, 2:3], a1[:],
                                   op0=ALU.mult, op1=ALU.add)
            a3 = T([P, 2, W], "cv_a1")
            E.scalar_tensor_tensor(a3[:], buf[:, :, 3:W + 3], wc[:, 3:4], a2[:],
                                   op0=ALU.mult, op1=ALU.add)
            xc = T([P, 2, W], tag)
            S.activation(xc[:], a3[:], AF.Silu, bias=cb[:, 0:1])
            return xc

        # ================= spa mamba =================
        def spa_mamba(i, xs):
            xflat = xs[:].rearrange("p s t -> p (s t)")
            xln = part_ln(xflat, i)
            tap(f"xln{i}", lambda: ([128, 512], lambda d: dma(d[:], xln[:])))
            inw_t = T([128, 644], "w_spa_in")
            dma(inw_t[:], w_t['spa_in_wT'][i])
            inw = inw_t[:]
            # dt chain first: keeps scalar engine in the ln/exp table while
            # part_ln's exp is still resident, before the silu block
            pdt = psS.tile([4, 512], F32, tag="small", name="small")
            MM(pdt[:], inw[:, 640:644], xln[:], start=True, stop=True)
            e1 = T([4, 512], "rowA")
            S.activation(e1[:], pdt[:], AF.Exp, bias=col(f"spa_dtb{i}", 4))
            # softplus via ln(1+u) Taylor (|u|<0.5): keeps scalar engine out
            # of the Ln table mid-silu-run
            u2 = T([4, 512], "rowB")
            V.tensor_mul(u2[:], e1[:], e1[:])
            u3 = T([4, 512], "tay3")
            V.tensor_mul(u3[:], u2[:], e1[:])
            u4 = T([4, 512], "tay4")
            V.tensor_mul(u4[:], u2[:], u2[:])
            u5 = T([4, 512], "tay5")
            V.tensor_mul(u5[:], u2[:], u3[:])
            d1 = T([4, 512], "tay6")
            V.scalar_tensor_tensor(d1[:], u2[:], -0.5, e1[:],
                                   op0=ALU.mult, op1=ALU.add)
            d2 = T([4, 512], "rowB")
            V.scalar_tensor_tensor(d2[:], u3[:], 1.0 / 3, d1[:],
                                   op0=ALU.mult, op1=ALU.add)
            d3 = T([4, 512], "tay3")
            V.scalar_tensor_tensor(d3[:], u4[:], -0.25, d2[:],
                                   op0=ALU.mult, op1=ALU.add)
            dtv = T([4, 512], "mb_dtv")
            V.scalar_tensor_tensor(dtv[:], u5[:], 0.2, d3[:],
                                   op0=ALU.mult, op1=ALU.add)
            dtA = T([4, 512], "rowA")
            V.tensor_scalar(dtA[:], dtv[:], col(f"spa_alog{i}", 4), -1.0,
                            op0=ALU.mult, op1=ALU.mult)
            acum = T([4, 512], "mb_acum")
            aflat = T([1, 2, 1024], "aflat")
            for s in range(BPC):
                V.tensor_tensor_scan(acum[:, s * 256:(s + 1) * 256],
                                     dtA[:, s * 256:(s + 1) * 256],
                                     dtA[:, s * 256:(s + 1) * 256], 0.0,
                                     op0=ALU.add, op1=ALU.bypass)
                dma(aflat[0:1, s, :].rearrange("o (p f) -> o p f", p=4),
                    acum[:, s * 256:(s + 1) * 256])
            # in_proj: z (2 blocks), x (2 blocks), B, C
            zsil = T([128, 2, 512], "mb_zsil")
            for j in range(2):
                pz = P512()
                MM(pz[:], inw[:, j * 128:(j + 1) * 128], xln[:],
                                 start=True, stop=True)
                S.activation(zsil[:, j, :], pz[:], AF.Silu)
            cvx = []
            for j in range(2):
                px = P512()
                MM(px[:], inw[:, 256 + j * 128:256 + (j + 1) * 128], xln[:],
                                 start=True, stop=True)
                buf = T([128, 2, 259], f"cv_x{j}")
                V.memset(buf[:, :, 0:3], 0.0)
                S.copy(buf[:, :, 3:259], px[:].rearrange("p (s t) -> p s t", s=2))
                cvx.append(buf)
            pbc = P512()
            MM(pbc[:], inw[:, 512:640], xln[:], start=True, stop=True)
            bufbc = T([128, 2, 259], "cv_B")
            V.memset(bufbc[:, :, 0:3], 0.0)
            S.copy(bufbc[:, :, 3:259], pbc[:].rearrange("p (s t) -> p s t", s=2))
            tap(f"dtv{i}", lambda: ([4, 512], lambda d: dma(d[:], dtv[:])))
            tap(f"acum{i}", lambda: ([4, 512], lambda d: dma(d[:], acum[:])))
            # conv + silu
            xc = []
            for j in range(2):
                xc.append(convchain(cvx[j], wt['spa_conv_pk'][:, i, j, :],
                                    col(f"spa_cb{i}_{j}"), 128, 256, f"xc_{j}"))
            xcBC = convchain(bufbc, wt['spa_conv_pk'][:, i, 2, :],
                             col(f"spa_cbBC{i}"), 128, 256, "xc_B")
            xcB = xcBC[0:64]
            xcC = T([64, 2, 256], "xc_C")
            dma(xcC[:], xcBC[64:128])
            if i == 0:
                tap("xc00", lambda: ([128, 512], lambda d: dma(
                    d[:], xc[0][:].rearrange("p s t -> p (s t)"))))
                tap("xcB0", lambda: ([64, 512], lambda d: dma(
                    d[:], xcB[:].rearrange("p s t -> p (s t)"))))
                tap("xcC0", lambda: ([64, 512], lambda d: dma(
                    d[:], xcC[:].rearrange("p s t -> p (s t)"))))
            # dt-scaled x (feature-major): xp[:, j, :] = xc[j] * dtB_j
            xp = T([128, 2, 512], "mb_xp")
            for j in range(2):
                pdb = P512()
                MM(pdb[:], ct['E_spaJ'][:, j, :], dtv[:], start=True, stop=True)
                V.tensor_mul(xp[:, j, :],
                             xc[j][:].rearrange("p s t -> p (s t)"), pdb[:])
            h1 = T([128, 2, 256], "h1")
            for s in range(BPC):
                # token-major dt-scaled x: xtm [t(128), st, hp(256)]
                xtm = T([128, 2, 256], "spa_xtm")
                for st in range(2):
                    for j in range(2):
                        ptr = P256()
                        ptr16 = ptr[:].bitcast(F16)
                        TR(
                            ptr16[:, 0:128],
                            xp[:, j, s * 256 + st * 128: s * 256 + (st + 1) * 128],
                            ct['identh'][:])
                        S.copy(xtm[:, st, j * 128:(j + 1) * 128], ptr16[:, 0:128])
                # masked M0^T per s-tile
                m0m = T([128, 2, 256], "ssd_m0m")
                for st in range(2):
                    pm0 = P256()
                    MM(pm0[:], xcB[:, s, st * 128:(st + 1) * 128],
                                     xcC[:, s, :], start=True, stop=True)
                    V.tensor_mul(m0m[:, st, :], pm0[:], ct['maskT_spa'][:, st, :])
                # Acum transposes + strided copy
                acumT = T([128, 2, 4], "spa_acumT")
                for tt in range(2):
                    ptr = P256()
                    TR(ptr[:, 0:4],
                                        acum[:, s * 256 + tt * 128: s * 256 + (tt + 1) * 128],
                                        ident[0:4, 0:4])
                    S.copy(acumT[:, tt, :], ptr[:, 0:4])
                pball = P512()
                MM(pball[:], ones32[0:1, :], aflat[:, s, 0:512],
                   start=True, stop=True)
                pbal2 = P512()
                MM(pbal2[:], ones32[0:1, :], aflat[:, s, 512:1024],
                   start=True, stop=True)
                # Y accumulation per head over s-tiles
                ypsl = [P256(), P256()]
                for st in range(2):
                    Dt = T([128, 4, 256], "ssd_Dt")
                    for h in range(H1):
                        pbx = pball if h < 2 else pbal2
                        V.tensor_scalar(Dt[:, h, :],
                                        pbx[:, (h % 2) * 256:(h % 2 + 1) * 256],
                                        acumT[:, st, h:h + 1], 0.0,
                                        op0=ALU.subtract, op1=ALU.min)
                    Et = T([128, 4, 256], "ssd_Et")
                    S.activation(Et[:].rearrange("p h t -> p (h t)"),
                                 Dt[:].rearrange("p h t -> p (h t)"), AF.Exp)
                    MT = T([128, 4, 256], "ssd_MT")
                    V.tensor_tensor(MT[:], Et[:],
                                    m0m[:, st, :].unsqueeze(1).to_broadcast((128, 4, 256)),
                                    op=ALU.mult)
                    if i == 0 and s == 0 and st == 0:
                        tap("Dt00", lambda: ([128, 1024], lambda d: dma(
                            d[:], Dt[:].rearrange("p h t -> p (h t)"))))
                        tap("MT00", lambda: ([128, 1024], lambda d: dma(
                            d[:], MT[:].rearrange("p h t -> p (h t)"))))
                    for h in range(H1):
                        MM(ypsl[h // 2][(h % 2) * 64:(h % 2) * 64 + 64, :],
                                         xtm[:, st, h * 64:(h + 1) * 64],
                                         MT[:, h, :],
                                         start=(st == 0), stop=(st == 1),
                                         tile_position=(0, (h % 2) * 64),
                                         skip_group_check=True)
                if i == 0 and s == 0:
                    tap("xtm0", lambda: ([128, 512], lambda d: dma(
                        d[:], xtm[:].rearrange("p s t -> p (s t)"))))
                    tap("m0m0", lambda: ([128, 512], lambda d: dma(
                        d[:], m0m[:].rearrange("p s t -> p (s t)"))))
                    tap("acumT0", lambda: ([128, 8], lambda d: dma(
                        d[:], acumT[:].rearrange("p s t -> p (s t)"))))
                    tap("acs0", lambda: ([128, 256], lambda d: dma(d[:], acs[:])))
                ygt = T([128, 2, 256], "spa_ygt")
                y0t = T([128, 2, 256], "spa_y0t")
                for j in range(2):
                    V.scalar_tensor_tensor(y0t[:, j, :], xc[j][:, s, :],
                                           col(f"spa_dpc{i}_{j}"),
                                           ypsl[j][:], op0=ALU.mult, op1=ALU.add)
                    V.tensor_mul(ygt[:, j, :], y0t[:, j, :],
                                 zsil[:, j, s * 256:(s + 1) * 256])
                if i == 0 and s == 0:
                    tap("y00", lambda: ([128, 512], lambda d: dma(
                        d[:], y0t[:].rearrange("p j t -> p (j t)"))))
                    tap("zsil0", lambda: ([128, 1024], lambda d: dma(
                        d[:], zsil[:].rearrange("p j t -> p (j t)"))))
                # gated RMS norm over d_inner
                sqy = T([128, 2, 256], "sqy16")
                S.activation(sqy[:].rearrange("p j t -> p (j t)"),
                             ygt[:].rearrange("p j t -> p (j t)"), AF.Square)
                ssy = psS.tile([1, 256], F32, tag="small", name="small")
                for j in range(2):
                    MM(ssy[:], ones4[:, 0:1], sqy[:, j, :],
                                     start=(j == 0), stop=(j == 1))
                rl = T([1, 256], "rowB")
                S.activation(rl[:], ssy[:], AF.Ln, bias=epscol[0:1, 0:1],
                             scale=1.0 / 256)
                rrow = T([1, 256], "rowC")
                S.activation(rrow[:], rl[:], AF.Exp, scale=-0.5)
                rB = P256()
                MM(rB[:], onesrow1, rrow[:], start=True, stop=True)
                ynt = T([128, 2, 256], "spa_ynt")
                for j in range(2):
                    V.scalar_tensor_tensor(ynt[:, j, :], ygt[:, j, :],
                                           col(f"spa_rwc{i}_{j}"),
                                           rB[:], op0=ALU.mult, op1=ALU.mult)
                if i == 0 and s == 0:
                    tap("ygt0", lambda: ([128, 512], lambda d: dma(
                        d[:], ygt[:].rearrange("p s t -> p (s t)"))))
                    tap("ynt0", lambda: ([128, 512], lambda d: dma(
                        d[:], ynt[:].rearrange("p s t -> p (s t)"))))
                pop = P256()
                for j in range(2):
                    MM(pop[:], wt['spa_out_pk'][:, i, j, :], ynt[:, j, :],
                                     start=(j == 0), stop=(j == 1))
                V.tensor_add(h1[:, s, :], pop[:], xs[:, s, :])
            return h1

        # ================= spe mamba =================
        def spe_mamba(i, h1):
            # LayerNorm over the 256 features (free dim), batched samples
            mus = T([128, 2], "spe_mus")
            V.tensor_reduce(mus[:], h1[:], axis=AX.X, op=ALU.add)
            sq2 = T([128, 512], "sq_tmp")
            S.activation(sq2[:], h1[:].rearrange("p s t -> p (s t)"), AF.Square)
            ss2 = T([128, 2], "spe_ss2")
            V.tensor_reduce(ss2[:], sq2[:].rearrange("p (s t) -> p s t", s=2),
                            axis=AX.X, op=ALU.add)
            mean = T([128, 2], "spe_mean")
            V.tensor_scalar(mean[:], mus[:], 1.0 / 256, None, op0=ALU.mult)
            m2 = T([128, 2], "spe_m2")
            V.tensor_mul(m2[:], mean[:], mean[:])
            var2 = T([128, 2], "spe_var")
            V.scalar_tensor_tensor(var2[:], ss2[:], 1.0 / 256, m2[:],
                                   op0=ALU.mult, op1=ALU.subtract)
            l2t = T([128, 2], "spe_l2")
            S.activation(l2t[:], var2[:], AF.Ln, bias=epscol[:, 0:1])
            rstd2 = T([128, 2], "spe_rstd")
            S.activation(rstd2[:], l2t[:], AF.Exp, scale=-0.5)
            X2f = T([128, 2, 2, 128], "x2f_tmp")
            for s in range(BPC):
                xn = T([128, 256], "spe_xn")
                V.tensor_scalar(xn[:], h1[:, s, :], mean[:, s:s + 1], rstd2[:, s:s + 1],
                                op0=ALU.subtract, op1=ALU.mult)
                u = T([128, 256], "spe_u")
                V.tensor_mul(u[:], xn[:], wt['spe_ln_wB'][:, i, :])
                xsn = T([128, 256], "spe_xsn")
                V.tensor_add(xsn[:], u[:], wt['spe_ln_bB'][:, i, :])
                for ft in range(2):
                    ptr = P256()
                    ptr16 = ptr[:].bitcast(F16)
                    TR(ptr16[:, 0:128], xsn[:, ft * 128:(ft + 1) * 128],
                       ct['identh'][:])
                    S.copy(X2f[:, s, ft, :], ptr16[:, 0:128])
            # in_proj (samples batched along free): out cols ordered (s, t2)
            inw2t = T([128, 2, 1160], "w_spe_in")
            dma(inw2t[:], w_t['spe_in_pk'][i])
            inw2 = inw2t[:]
            ow2t = T([128, 4, 256], "w_spe_out")
            dma(ow2t[:], w_t['spe_out_pk'][i])
            ow2 = ow2t[:]

            def mm2(out_ap, off, width):
                for k in range(2):
                    MM(out_ap,
                                     inw2[:, k, off:off + width],
                                     X2f[:, :, k, :],
                                     start=(k == 0), stop=(k == 1))
            pdt = psS.tile([8, 256], F32, tag="small", name="small")
            for k in range(2):
                MM(pdt[:], inw2[:, k, 1152:1160],
                                 X2f[:, :, k, :], start=(k == 0), stop=(k == 1))
            e1 = T([8, 256], "rowA")
            S.activation(e1[:], pdt[:], AF.Exp, bias=col(f"spe_dtb{i}", 8))
            u2 = T([8, 256], "rowB")
            V.tensor_mul(u2[:], e1[:], e1[:])
            u3 = T([8, 256], "tay3")
            V.tensor_mul(u3[:], u2[:], e1[:])
            u4 = T([8, 256], "tay4")
            V.tensor_mul(u4[:], u2[:], u2[:])
            u5 = T([8, 256], "tay5")
            V.tensor_mul(u5[:], u2[:], u3[:])
            d1 = T([8, 256], "tay6")
            V.scalar_tensor_tensor(d1[:], u2[:], -0.5, e1[:],
                                   op0=ALU.mult, op1=ALU.add)
            d2 = T([8, 256], "rowB")
            V.scalar_tensor_tensor(d2[:], u3[:], 1.0 / 3, d1[:],
                                   op0=ALU.mult, op1=ALU.add)
            d3 = T([8, 256], "tay3")
            V.scalar_tensor_tensor(d3[:], u4[:], -0.25, d2[:],
                                   op0=ALU.mult, op1=ALU.add)
            dtv = T([8, 256], "mb_dtv")
            V.scalar_tensor_tensor(dtv[:], u5[:], 0.2, d3[:],
                                   op0=ALU.mult, op1=ALU.add)
            dtA = T([8, 256], "rowA")
            V.tensor_scalar(dtA[:], dtv[:], col(f"spe_alog{i}", 8), -1.0,
                            op0=ALU.mult, op1=ALU.mult)
            acum = T([8, 256], "mb_acum")
            aflat = T([1, 2, 1024], "aflat")
            for s in range(BPC):
                V.tensor_tensor_scan(acum[:, s * 128:(s + 1) * 128],
                                     dtA[:, s * 128:(s + 1) * 128],
                                     dtA[:, s * 128:(s + 1) * 128], 0.0,
                                     op0=ALU.add, op1=ALU.bypass)
                dma(aflat[0:1, s, :].rearrange("o (p f) -> o p f", p=8),
                    acum[:, s * 128:(s + 1) * 128])
            z2sil = T([128, 4, 256], "mb_zsil")
            for j in range(4):
                pz = P256()
                mm2(pz[:], j * 128, 128)
                S.activation(z2sil[:, j, :], pz[:], AF.Silu)
            cvx2 = []
            for j in range(4):
                px = P256()
                mm2(px[:], 512 + j * 128, 128)
                buf = T([128, 2, 131], f"cv_x{j}")
                V.memset(buf[:, :, 0:3], 0.0)
                S.copy(buf[:, :, 3:131], px[:].rearrange("p (s t) -> p s t", s=2))
                cvx2.append(buf)
            pbc = P256()
            for k in range(2):
                MM(pbc[:], inw2[:, k, 1024:1152],
                   X2f[:, :, k, :], start=(k == 0), stop=(k == 1))
            bufbc = T([128, 2, 131], "cv_B")
            V.memset(bufbc[:, :, 0:3], 0.0)
            S.copy(bufbc[:, :, 3:131], pbc[:].rearrange("p (s t) -> p s t", s=2))
            # conv + silu
            xc2 = []
            for j in range(4):
                xc2.append(convchain(cvx2[j], wt['spe_conv_pk'][:, i, j, :],
                                     col(f"spe_cb{i}_{j}"), 128, 128, f"xc_{j}"))
            xcBC = convchain(bufbc, wt['spe_conv_pk'][:, i, 4, :],
                             col(f"spe_cbBC{i}"), 128, 128, "xc_B")
            xcB = xcBC[0:64]
            xcC = T([64, 2, 128], "xc_C")
            dma(xcC[:], xcBC[64:128])
            # dt-scaled x
            xp2 = T([128, 4, 256], "mb_xp")
            for j in range(4):
                pdb = P256()
                MM(pdb[:], ct['E_speJ'][:, j, :], dtv[:], start=True, stop=True)
                V.tensor_mul(xp2[:, j, :],
                             xc2[j][:].rearrange("p s t -> p (s t)"), pdb[:])
            xs_new = T([128, 2, 256], "xs")
            for s in range(BPC):
                xtm2 = T([128, 512], "spe_xtm")
                for j in range(4):
                    ptr = P256()
                    ptr16 = ptr[:].bitcast(F16)
                    TR(ptr16[:, 0:128],
                       xp2[:, j, s * 128:(s + 1) * 128], ct['identh'][:])
                    S.copy(xtm2[:, j * 128:(j + 1) * 128], ptr16[:, 0:128])
                m0m2 = T([128, 128], "ssd_m0m")
                pm0 = P256()
                MM(pm0[:, 0:128], xcB[:, s, :], xcC[:, s, :],
                                 start=True, stop=True)
                V.tensor_mul(m0m2[:], pm0[:, 0:128], ct['maskT_spe'][:])
                acumT = T([128, 8], "spe_acumT")
                ptr = P256()
                TR(ptr[:, 0:8], acum[:, s * 128:(s + 1) * 128],
                                    ident[0:8, 0:8])
                S.copy(acumT[:], ptr[:, 0:8])
                pball = P512()
                MM(pball[:], ones32[0:1, :], aflat[:, s, 0:512],
                   start=True, stop=True)
                pbal2 = P512()
                MM(pbal2[:], ones32[0:1, :], aflat[:, s, 512:1024],
                   start=True, stop=True)
                Dt = T([128, 8, 128], "ssd_Dt")
                for h in range(H2):
                    pbx = pball if h < 4 else pbal2
                    V.tensor_scalar(Dt[:, h, :],
                                    pbx[:, (h % 4) * 128:(h % 4 + 1) * 128],
                                    acumT[:, h:h + 1], 0.0,
                                    op0=ALU.subtract, op1=ALU.min)
                Et = T([128, 8, 128], "ssd_Et")
                S.activation(Et[:].rearrange("p h t -> p (h t)"),
                             Dt[:].rearrange("p h t -> p (h t)"), AF.Exp)
                MT = T([128, 8, 128], "ssd_MT")
                V.tensor_tensor(MT[:], Et[:],
                                m0m2[:].unsqueeze(1).to_broadcast((128, 8, 128)),
                                op=ALU.mult)
                ygt2 = T([128, 4, 128], "spe_ygt")
                for j in range(4):
                    yp = P256()
                    for hh in range(2):
                        h = 2 * j + hh
                        MM(yp[hh * 64:hh * 64 + 64, 0:128],
                                         xtm2[:, h * 64:(h + 1) * 64],
                                         MT[:, h, :], start=True, stop=True,
                                         tile_position=(0, hh * 64),
                                         skip_group_check=True)
                    y0 = T([128, 128], "spe_y0")
                    V.scalar_tensor_tensor(y0[:], xc2[j][:, s, :],
                                           col(f"spe_dpc{i}_{j}"),
                                           yp[:, 0:128], op0=ALU.mult, op1=ALU.add)
                    V.tensor_mul(ygt2[:, j, :], y0[:],
                                 z2sil[:, j, s * 128:(s + 1) * 128])
                sqy = T([128, 4, 128], "sqy16")
                S.activation(sqy[:].rearrange("p j t -> p (j t)"),
                             ygt2[:].rearrange("p j t -> p (j t)"), AF.Square)
                ssy = psS.tile([1, 128], F32, tag="small", name="small")
                for j in range(4):
                    MM(ssy[:], ones4[:, 0:1], sqy[:, j, :],
                                     start=(j == 0), stop=(j == 3))
                rl = T([1, 128], "rowB")
                S.activation(rl[:], ssy[:], AF.Ln, bias=epscol[0:1, 0:1],
                             scale=1.0 / 512)
                rrow = T([1, 128], "rowC")
                S.activation(rrow[:], rl[:], AF.Exp, scale=-0.5)
                rB = P256()
                MM(rB[:, 0:128], onesrow1, rrow[:], start=True, stop=True)
                ynt = T([128, 4, 128], "spe_ynt")
                for j in range(4):
                    V.scalar_tensor_tensor(ynt[:, j, :], ygt2[:, j, :],
                                           col(f"spe_rwc{i}_{j}"),
                                           rB[:, 0:128], op0=ALU.mult, op1=ALU.mult)
                for ft in range(2):
                    ph2 = P256()
                    for k in range(4):
                        MM(ph2[:, 0:128],
                                         ow2[:, k, ft * 128:(ft + 1) * 128],
                                         ynt[:, k, :], start=(k == 0), stop=(k == 3))
                    h2sb = T([128, 128], "spe_h2sb")
                    S.copy(h2sb[:], ph2[:, 0:128])
                    ptr = P256()
                    ptr16 = ptr[:].bitcast(F16)
                    TR(ptr16[:, 0:128], h2sb[:], ct['identh'][:])
                    V.tensor_add(xs_new[:, s, ft * 128:(ft + 1) * 128],
                                 ptr16[:, 0:128],
                                 h1[:, s, ft * 128:(ft + 1) * 128])
            return xs_new

        # ================= layers =================
        cur = xs
        for i in range(2):
            h1 = spa_mamba(i, cur)
            tap(f"h1_{i}", lambda: tap_batched(h1, [128, L]))
            cur = spe_mamba(i, h1)
            tap(f"xsl{i + 1}", lambda: tap_batched(cur, [128, L]))

        load_w(['cprj_pk', 'aqT', 'akT', 'avT', 'aoT', 'sqT', 'skT', 'svT',
                'soT', 'svbB', 'sobB', 'dsw_pk', 'ds_ln_wB', 'ds_ln_bB'])

        # ================= final LN =================
        xfl = part_ln(cur[:].rearrange("p s t -> p (s t)"), 2)
        xf = xfl[:].rearrange("p (s t) -> p s t", s=BPC)
        tap("xf", lambda: ([BPC, 128, L],
                           lambda d: [dma(d[s], xf[:, s, :]) for s in range(BPC)]))

        # ================= spa attention (center query) =================
        pctr = psS.tile([128, 2], F32, tag="small", name="small")
        for l in range(5):
            MM(pctr[:], wt['cprj_pk'][:, l, :], xf[:, :, l],
                             start=(l == 0), stop=(l == 4))
        ctr = T([128, 2], "at_ctr")
        S.activation(ctr[:], pctr[:], AF.Identity, bias=col("cprj_b"))
        pq = psS.tile([128, 2], F32, tag="small", name="small")
        MM(pq[:], wt['aqT'][:], ctr[:], start=True, stop=True)
        qsb = T([128, 2], "at_q")
        S.activation(qsb[:], pq[:], AF.Identity, bias=col("aq_b"))
        pk = P512()
        MM(pk[:], wt['akT'][:], xfl[:], start=True, stop=True)
        Ksb = T([128, 2, 256], "at_K")
        S.activation(Ksb[:].rearrange("p s t -> p (s t)"), pk[:], AF.Identity,
                     bias=col("ak_b"))
        pv = P512()
        MM(pv[:], wt['avT'][:], xfl[:], start=True, stop=True)
        Vsb = T([128, 2, 256], "at_V")
        S.activation(Vsb[:].rearrange("p s t -> p (s t)"), pv[:], AF.Identity,
                     bias=col("av_b"))
        vo = T([128, 2, 256], "at_vo")
        for s in range(BPC):
            qd = T([128, 8], "at_qd")
            V.tensor_tensor(qd[:], qsb[:, s:s + 1].to_broadcast((128, 8)),
                            ct['Emask_q'][:], op=ALU.mult)
            plg = psS.tile([8, 256], F32, tag="small", name="small")
            MM(plg[:], qd[:], Ksb[:, s, :], start=True, stop=True)
            nm = T([8, 1], "at_nm")
            V.tensor_reduce(nm[:], plg[:], axis=AX.X, op=ALU.max, negate=True)
            nm4 = T([8, 1], "at_nm4")
            V.tensor_scalar(nm4[:], nm[:], 0.25, None, op0=ALU.mult)
            ex = T([8, 256], "at_ex")
            S.activation(ex[:], plg[:], AF.Exp, bias=nm4[:, 0:1], scale=0.25)
            sm = T([8, 1], "at_sm")
            V.tensor_reduce(sm[:], ex[:], axis=AX.X, op=ALU.add)
            rc = T([8, 1], "at_rc")
            V.reciprocal(rc[:], sm[:])
            aw = T([8, 256], "at_aw")
            V.tensor_scalar(aw[:], ex[:], rc[:, 0:1], None, op0=ALU.mult)
            patB = P256()
            MM(patB[:], ct['E_attn'][:], aw[:], start=True, stop=True)
            V.tensor_mul(vo[:, s, :], Vsb[:, s, :], patB[:])
        pao = P512()
        MM(pao[:], wt['aoT'][:], vo[:].rearrange("p s t -> p (s t)"),
                         start=True, stop=True)
        xa = T([128, 2, 256], "xa")
        V.scalar_tensor_tensor(xa[:].rearrange("p s t -> p (s t)"), pao[:],
                               col("ao_b"), xfl[:], op0=ALU.add, op1=ALU.add)
        tap("xa", lambda: tap_batched(xa, [128, L]))

        # ================= spe attention =================
        X2a = T([128, 2, 2, 128], "x2f_tmp")
        for s in range(BPC):
            for ft in range(2):
                ptr = P256()
                TR(ptr[:, 0:128], xa[:, s, ft * 128:(ft + 1) * 128],
                                    ident[:])
                S.copy(X2a[:, s, ft, :], ptr[:, 0:128])
        q2 = T([128, 2, 2, 128], "sp2_q2")
        k2 = T([128, 2, 2, 128], "sp2_k2")
        for ot in range(2):
            pq2 = P256()
            for ft in range(2):
                MM(pq2[:], wt['sqT'][:, ft, ot * 128:(ot + 1) * 128],
                   X2a[:, :, ft, :], start=(ft == 0), stop=(ft == 1))
            for s in range(BPC):
                S.activation(q2[:, s, ot, :], pq2[:, s * 128:(s + 1) * 128],
                             AF.Identity, bias=col(f"sq_b{ot}"))
            pk2 = P256()
            for ft in range(2):
                MM(pk2[:], wt['skT'][:, ft, ot * 128:(ot + 1) * 128],
                   X2a[:, :, ft, :], start=(ft == 0), stop=(ft == 1))
            for s in range(BPC):
                S.activation(k2[:, s, ot, :], pk2[:, s * 128:(s + 1) * 128],
                             AF.Identity, bias=col(f"sk_b{ot}"))
        xs2 = T([128, 2, 256], "xs2")
        for s in range(BPC):
            pv2 = P256()
            for ft in range(2):
                MM(pv2[:], X2a[:, s, ft, :], wt['svT'][:, ft, :],
                                 start=(ft == 0), stop=(ft == 1))
            v2 = T([128, 256], "sp2_v2")
            V.tensor_add(v2[:], pv2[:], wt['svbB'][:])
            pa2 = P256()
            for ot in range(2):
                MM(pa2[:, 0:128], q2[:, s, ot, :], k2[:, s, ot, :],
                                 start=(ot == 0), stop=(ot == 1))
            nm = T([128, 1], "sp2_nm")
            V.tensor_reduce(nm[:], pa2[:, 0:128], axis=AX.X, op=ALU.max, negate=True)
            nm16 = T([128, 1], "sp2_nm16")
            V.tensor_scalar(nm16[:], nm[:], 1.0 / 16, None, op0=ALU.mult)
            ex = T([128, 128], "sp2_ex")
            S.activation(ex[:], pa2[:, 0:128], AF.Exp, bias=nm16[:, 0:1], scale=1.0 / 16)
            sm = T([128, 1], "sp2_sm")
            V.tensor_reduce(sm[:], ex[:], axis=AX.X, op=ALU.add)
            rc = T([128, 1], "sp2_rc")
            V.reciprocal(rc[:], sm[:])
            a2 = T([128, 128], "sp2_a2")
            V.tensor_scalar(a2[:], ex[:], rc[:, 0:1], None, op0=ALU.mult)
            pa2T = P256()
            pa2T16 = pa2T[:].bitcast(F16)
            TR(pa2T16[:, 0:128], a2[:], ct['identh'][:])
            a2T = T([128, 128], "sp2_a2T")
            S.copy(a2T[:], pa2T16[:, 0:128])
            o2 = T([128, 2, 128], "sp2_o2")
            for ot in range(2):
                po2 = P256()
                MM(po2[:, 0:128], v2[:, ot * 128:(ot + 1) * 128], a2T[:],
                                 start=True, stop=True)
                S.copy(o2[:, ot, :], po2[:, 0:128])
            po3 = P256()
            for ot in range(2):
                MM(po3[:], o2[:, ot, :], wt['soT'][:, ot, :],
                                 start=(ot == 0), stop=(ot == 1))
            t3 = T([128, 256], "sp2_t3")
            V.tensor_add(t3[:], po3[:], wt['sobB'][:])
            V.tensor_add(xs2[:, s, :], t3[:], xa[:, s, :])
        tap("xs2", lambda: tap_batched(xs2, [128, L]))

        # ================= downsample =================
        pds = psD.tile([64, 256], F32, tag="ds", name="ds")
        invr = T([1, BPC, L], "irow_raw", I32)
        dma(invr[:], inv[None, :, :])
        invf = T([1, BPC, L], "irow_f")
        V.tensor_copy(invf[:], invr[:])
        for s in range(BPC):
            # inverse permutation (argsort-based) one-hot
            invB = P512()
            MM(invB[:, 0:L], onesrow1, invf[:, s, :], start=True, stop=True)
            QT = T([128, 2, 256], "perm_oh")
            for tt in range(2):
                V.tensor_scalar(QT[:, tt, :], invB[:, 0:L],
                                ct['iotaC'][:, tt:tt + 1], None,
                                op0=ALU.is_equal)
            tmv = T([128, 2, 128], "tm_tmp")
            for tt in range(2):
                ptr = P256()
                ptr16 = ptr[:].bitcast(F16)
                TR(ptr16[:, 0:128], xs2[:, s, tt * 128:(tt + 1) * 128],
                   ct['identh'][:])
                S.copy(tmv[:, tt, :], ptr16[:, 0:128])
            pxr = P256()
            for tt in range(2):
                MM(pxr[:], tmv[:, tt, :], QT[:, tt, :],
                                 start=(tt == 0), stop=(tt == 1))
            xrp = T([128, 324], "ds_xrp")
            V.memset(xrp[:], 0.0)
            xr3 = xrp[:].rearrange("p (h w) -> p h w", h=18)
            S.copy(xr3[:, 1:17, 1:17], pxr[:].rearrange("p (h w) -> p h w", h=16))
            for kh in range(3):
                for kw in range(3):
                    k = kh * 3 + kw
                    cmp_ = T([128, 64], "ds_cmp")
                    V.tensor_copy(cmp_[:].rearrange("p (a b) -> p a b", a=8),
                                  xr3[:, kh:kh + 16:2, kw:kw + 16:2])
                    MM(pds[:, s * 128:(s + 1) * 128],
                                     cmp_[:],
                                     wt['dsw_pk'][:, k, :],
                                     start=(k == 0), stop=(k == 8),
                                     skip_group_check=True)
        view2 = pds[:].rearrange("p (s c) -> p s c", s=2)
        mus = T([64, 2], "ds_mus")
        V.tensor_reduce(mus[:], view2, axis=AX.X, op=ALU.add)
        mean = T([64, 2], "ds_mean")
        V.tensor_scalar(mean[:], mus[:], 1.0 / 128, None, op0=ALU.mult)
        sq = T([64, 2, 128], "sq_tmp")
        S.activation(sq[:].rearrange("p s c -> p (s c)"), pds[:], AF.Square)
        ss = T([64, 2], "ds_ss")
        V.tensor_reduce(ss[:], sq[:], axis=AX.X, op=ALU.add)
        m2 = T([64, 2], "ds_m2")
        V.tensor_mul(m2[:], mean[:], mean[:])
        var = T([64, 2], "ds_var")
        V.scalar_tensor_tensor(var[:], ss[:], 1.0 / 128, m2[:],
                               op0=ALU.mult, op1=ALU.subtract)
        lv = T([64, 2], "ds_lv")
        S.activation(lv[:], var[:], AF.Ln, bias=epscol[0:64, 0:1])
        rstd = T([64, 2], "ds_rstd")
        S.activation(rstd[:], lv[:], AF.Exp, scale=-0.5)
        xn = T([64, 2, 128], "ds_xn")
        V.tensor_tensor(xn[:], view2,
                        mean[:].unsqueeze(2).to_broadcast((64, 2, 128)),
                        op=ALU.subtract)
        xr2 = T([64, 2, 128], "ds_t1")
        V.tensor_tensor(xr2[:], xn[:],
                        rstd[:].unsqueeze(2).to_broadcast((64, 2, 128)),
                        op=ALU.mult)
        o1 = T([64, 2, 128], "ds_o1")
        V.tensor_tensor(o1[:], xr2[:],
                        wt['ds_ln_wB'][:].unsqueeze(1).to_broadcast((64, 2, 128)),
                        op=ALU.mult)
        o2 = T([64, 2, 128], "ds_xn")
        V.tensor_tensor(o2[:], o1[:],
                        wt['ds_ln_bB'][:].unsqueeze(1).to_broadcast((64, 2, 128)),
                        op=ALU.add)
        for s in range(BPC):
            dma(out[s].rearrange("h w c -> (h w) c"), o2[:, s, :])

        stk.close()
    return nc, tap_t


# ---------------------------------------------------------------------------
_CACHE = {}


def _get_program(taps=()):
    key = tuple(sorted(taps))
    if key not in _CACHE:
        _CACHE[key] = build_program(taps)
    return _CACHE[key]


def make_inmaps(inputs, taps=()):
    cst = host_constants()
    w = prep_weights(inputs)
    x = np.asarray(inputs['x'], np.float32).reshape(16, C, L)
    idx = np.asarray(inputs['sorted_index'], np.int32)
    inv = np.argsort(idx, axis=1, kind='stable').astype(np.int32)
    in_maps = []
    for c in range(NCORES):
        m = {}
        m.update({k: np.ascontiguousarray(v) for k, v in cst.items()})
        m.update({k: np.ascontiguousarray(v) for k, v in w.items()})
        sl = slice(c * BPC, (c + 1) * BPC)
        m['x2'] = np.ascontiguousarray(x[sl])
        m['idx'] = np.ascontiguousarray(idx[sl])
        m['inv'] = np.ascontiguousarray(inv[sl])
        in_maps.append(m)
    return in_maps


def run(inputs, taps=(), trace=False):
    nc, tap_t = _get_program(taps)
    in_maps = make_inmaps(inputs, taps)
    res = run_bass_kernel_spmd(nc, in_maps, list(range(NCORES)), trace=trace)
    outs = np.concatenate([r['out'] for r in res.results], axis=0)
    tapd = {}
    for name in taps:
        tapd[name] = [r.get('t_' + name) for r in res.results]
    return outs, tapd, res


def kernel(**inputs):
    outs, _, _ = run(inputs)
    return outs

